# revision 10
# baseline (speedup 1.0000x reference)
"""CartBasisStressHead kernel for Trainium2 (8 NeuronCores, SPMD data-parallel).

Strategy
--------
Only 6 of the 9 m-rows of node_embedding are used: row 0 feeds a SiLU MLP
(per-node scalar), rows 4:9 feed a per-channel contraction (l=2 branch).
Nodes are sharded contiguously across 8 cores; segment sums are linear, so
the host adds per-shard partials.

The kernel is HBM-bound, so everything streamed is wired in fp8 (E4M3):
  * l=2 data uses a chain-of-4 compensated quantization (each node's rounding
    residual is folded into the next node of the same graph before rounding),
    cutting the segment-sum quantization error ~2x below plain fp8 rounding.
  * The segment sum itself runs on the PE in DoubleRow fp8 perf mode:
    256 nodes per pass (128 partitions x 2 pair lanes), with a 0/1 indicator
    matrix A[node, local_graph] as the stationary operand. DoubleRow only
    supports PSUM quadrant 0, so the two 320-column halves of the l=2
    features accumulate into two separate PSUM tiles.
  * The MLP runs x0 (fp8) against bf16 weights; per-node scalars come from a
    1-wide W3 matmul packed into spare PE column bands.

Inputs stream as 5-group superchunks, each split into a 1-group head DMA
(fast pipeline ramp) plus a 4-group rest DMA (~2.6 MB, near-peak HBM
efficiency); outputs stage in SBUF and store once per superchunk. Per-group
PE issue order interleaves the l=2 passes around the MLP layers so the
in-order PE queue never waits on the activation engine.

Host epilogue: scatter-add of per-group segment partials, contraction with
w_l2, bincount of per-node scalars, and the tiny (G,9)@(9,9) basis change.
"""

import sys

if "/opt/trn_rl_repo" not in sys.path:
    sys.path.insert(0, "/opt/trn_rl_repo")

import numpy as np
import ml_dtypes

import concourse.bacc as bacc
import concourse.tile as tile
from concourse import mybir
from concourse import bass_utils

_S2 = 2.0 ** -0.5
_S3 = 3.0 ** -0.5
_S6 = 6.0 ** -0.5
_CG = np.array([
    [_S3, 0, 0, 0, _S3, 0, 0, 0, _S3],
    [0, 0, 0, 0, 0, _S2, 0, -_S2, 0],
    [0, 0, -_S2, 0, 0, 0, _S2, 0, 0],
    [0, _S2, 0, -_S2, 0, 0, 0, 0, 0],
    [0, 0, _S2, 0, 0, 0, _S2, 0, 0],
    [0, 0, 0, 0, 0, _S2, 0, _S2, 0],
    [-_S6, 0, 0, 0, 2 * _S6, 0, 0, 0, -_S6],
    [0, _S2, 0, _S2, 0, 0, 0, 0, 0],
    [-_S2, 0, 0, 0, 0, 0, 0, 0, _S2],
], dtype=np.float32)  # (9, 9)

N_CORES = 8
P = 128          # SBUF partitions
NG = 1024        # nodes per group (one PSUM accumulation span)
DT = 4           # 256-node dtiles per group (DoubleRow pairs 2 nodes/lane)
ML2 = 640        # l=2 values per node (5 m-rows x 128 channels)
SCG = 5          # groups per superchunk (one input DMA / output store)
GPB = 5120       # el2 bytes per partition per group

F32 = mybir.dt.float32
BF16 = mybir.dt.bfloat16
F8 = mybir.dt.float8e4
DR = mybir.MatmulPerfMode.DoubleRow
WIRE8 = ml_dtypes.float8_e4m3

_BUILD_CACHE = {}


def _build(n_groups, W, n_real):
    key = (n_groups, W, n_real)
    if key in _BUILD_CACHE:
        return _BUILD_CACHE[key]

    n_sc = (n_groups + SCG - 1) // SCG
    T2 = n_groups * DT * 2

    nc = bacc.Bacc("TRN2", target_bir_lowering=False, debug=False,
                   num_devices=N_CORES)

    # inputs (host pre-packed; see kernel() for layouts)
    el2w = nc.dram_tensor("el2w", (n_sc, P, SCG * GPB), F8,
                          kind="ExternalInput").ap()
    x0w = nc.dram_tensor("x0w", (n_sc, P, SCG * NG), F8,
                         kind="ExternalInput").ap()
    lgid = nc.dram_tensor("lgid", (P, T2), F32, kind="ExternalInput").ap()
    iota_in = nc.dram_tensor("iota_in", (P, W), F32, kind="ExternalInput").ap()
    w1t = nc.dram_tensor("w1t", (P, P), BF16, kind="ExternalInput").ap()
    w2t = nc.dram_tensor("w2t", (P, P), BF16, kind="ExternalInput").ap()
    w3t = nc.dram_tensor("w3t", (P, 1), BF16, kind="ExternalInput").ap()
    b1 = nc.dram_tensor("b1c", (P, 1), F32, kind="ExternalInput").ap()
    b2 = nc.dram_tensor("b2c", (P, 1), F32, kind="ExternalInput").ap()
    # outputs
    scal = nc.dram_tensor("scal", (n_sc, SCG * NG), F32,
                          kind="ExternalOutput").ap()
    S_out = nc.dram_tensor("S_out", (n_sc, 32, SCG * 640), F32,
                           kind="ExternalOutput").ap()

    silu = mybir.ActivationFunctionType.Silu
    eq = mybir.AluOpType.is_equal

    with tile.TileContext(nc) as tc:
        with (
            tc.tile_pool(name="const", bufs=1) as cpool,
            tc.tile_pool(name="el2h", bufs=3) as el2hp,
            tc.tile_pool(name="el2r", bufs=3) as el2rp,
            tc.tile_pool(name="x0p", bufs=2) as x0p,
            tc.tile_pool(name="hp", bufs=4) as hp,
            tc.tile_pool(name="stp", bufs=2) as stp,
            tc.tile_pool(name="ph1", bufs=2, space="PSUM") as ph1p,
            tc.tile_pool(name="ph2", bufs=1, space="PSUM") as ph2p,
            tc.tile_pool(name="pS", bufs=1, space="PSUM") as pSp,
        ):
            w1s = cpool.tile([P, P], BF16)
            w2s = cpool.tile([P, P], BF16)
            w3s = cpool.tile([P, 1], BF16)
            b1s = cpool.tile([P, 1], F32)
            b2s = cpool.tile([P, 1], F32)
            iotas = cpool.tile([P, W], F32)
            lgids = cpool.tile([P, T2], F32)
            # lgid/iota ride the sync queue ahead of the bulk el2 streams so
            # the indicator build never waits behind megabytes of input
            nc.sync.dma_start(out=lgids[:], in_=lgid)
            nc.sync.dma_start(out=iotas[:], in_=iota_in)
            nc.scalar.dma_start(out=w1s[:], in_=w1t)
            nc.scalar.dma_start(out=w2s[:], in_=w2t)
            nc.scalar.dma_start(out=w3s[:], in_=w3t)
            nc.scalar.dma_start(out=b1s[:], in_=b1)
            nc.scalar.dma_start(out=b2s[:], in_=b2)

            # all indicator pair-matrices up front (one DVE instruction):
            # Aall[p, (g,d,i), w] = (iota[w] == lgid[p, (g,d,i)])
            Aall = cpool.tile([P, T2 * W], F8)
            nc.vector.tensor_tensor(
                out=Aall[:].rearrange("p (t w) -> p t w", t=T2, w=W),
                in0=iotas[:].unsqueeze(1).to_broadcast([P, T2, W]),
                in1=lgids[:].unsqueeze(2).to_broadcast([P, T2, W]),
                op=eq)

            for sc in range(n_sc):
                sc_g0 = sc * SCG
                sc_ng = min(SCG, n_groups - sc_g0)
                last_g = sc_g0 + sc_ng - 1
                lg_real = min(NG, n_real - last_g * NG)
                dt_last = (lg_real + 255) // 256
                sr_last = (lg_real + 511) // 512

                # head = group 0 of the superchunk (fast ramp), rest = tail
                el2h = el2hp.tile([P, GPB], F8, tag="el2h")
                nc.sync.dma_start(
                    out=el2h[:, :GPB if sc_ng > 1 else dt_last * 1280],
                    in_=el2w[sc][:, :GPB if sc_ng > 1 else dt_last * 1280])
                el2r = el2rp.tile([P, (SCG - 1) * GPB], F8, tag="el2r")
                if sc_ng > 1:
                    ext = (sc_ng - 2) * GPB + dt_last * 1280
                    nc.sync.dma_start(out=el2r[:, :ext],
                                      in_=el2w[sc][:, GPB: GPB + ext])
                x0c = x0p.tile([P, SCG * NG], F8, tag="x0c")
                xext = (sc_ng - 1) * NG + sr_last * 512
                nc.scalar.dma_start(out=x0c[:, :xext],
                                    in_=x0w[sc][:, :xext])

                scst = stp.tile([1, SCG * NG], F32, tag="scst")
                Sst = stp.tile([32, SCG * 640], F32, tag="Sst")

                for gl in range(sc_ng):
                    g = sc_g0 + gl
                    grp_real = min(NG, n_real - g * NG)
                    Sr = (grp_real + 511) // 512
                    Dr = (grp_real + 255) // 256
                    if gl == 0:
                        esrc = el2h
                        ebase = 0
                    else:
                        esrc = el2r
                        ebase = (gl - 1) * GPB

                    # one 2-bank PSUM tile per group:
                    #   [0:W, 0:320]    l=2 feature half 0 (bank A)
                    #   [0:W, 512:832]  l=2 feature half 1 (bank B)
                    #   [64:65, 0:512]  W3 scalars chunk 0
                    #   [96:97, 0:512]  W3 scalars chunk 1
                    pS = pSp.tile([P, 1024], F32, tag="pS")

                    def l2mm(d):
                        t2i = (g * DT + d) * 2
                        Ad = Aall[:, t2i * W: (t2i + 2) * W] \
                            .rearrange("p (i w) -> p i w", i=2, w=W)
                        base = ebase + d * 1280
                        nc.tensor.matmul(
                            pS[0:W, 0:320], Ad,
                            esrc[:, base: base + 640]
                                .rearrange("p (i f) -> p i f", i=2, f=320),
                            start=(d == 0), stop=(d == Dr - 1),
                            perf_mode=DR, tile_position=(0, 0))
                        nc.tensor.matmul(
                            pS[0:W, 512:832], Ad,
                            esrc[:, base + 640: base + 1280]
                                .rearrange("p (i f) -> p i f", i=2, f=320),
                            start=(d == 0), stop=(d == Dr - 1),
                            perf_mode=DR, tile_position=(0, 0))

                    # ---- MLP layer 1 (both chunks into one 2-bank tile) ----
                    h1p = ph1p.tile([P, 1024], F32, tag="h1p")
                    for s in range(Sr):
                        nsl = slice(gl * NG + s * 512, gl * NG + (s + 1) * 512)
                        nc.tensor.matmul(h1p[:, s * 512:(s + 1) * 512],
                                         w1s[:], x0c[:, nsl],
                                         start=True, stop=True)
                    h1s = hp.tile([P, 1024], BF16, tag="h1s")
                    nc.scalar.activation(h1s[:, :Sr * 512],
                                         h1p[:, :Sr * 512], silu, bias=b1s[:])

                    # ---- l=2 segment sum, first half of the dtiles ----
                    for d in range((Dr + 1) // 2):
                        l2mm(d)

                    # ---- MLP layer 2 (h1 ready by now; one batched silu) ----
                    h2p = ph2p.tile([P, 1024], F32, tag="h2p")
                    for s in range(Sr):
                        nc.tensor.matmul(h2p[:, s * 512:(s + 1) * 512],
                                         w2s[:], h1s[:, s * 512:(s + 1) * 512],
                                         start=True, stop=True)
                    h2s = hp.tile([P, 1024], BF16, tag="h2s")
                    nc.scalar.activation(h2s[:, :Sr * 512],
                                         h2p[:, :Sr * 512], silu, bias=b2s[:])

                    # ---- l=2 segment sum, second half ----
                    for d in range((Dr + 1) // 2, Dr):
                        l2mm(d)

                    # ---- per-node scalar: W3 @ h2, packed in col bands ----
                    for s in range(Sr):
                        q = 64 + 32 * s
                        nc.tensor.matmul(pS[q:q + 1, 0:512], w3s[:],
                                         h2s[:, s * 512:(s + 1) * 512],
                                         start=True, stop=True,
                                         tile_position=(0, q))

                    # ---- stage results in SBUF ----
                    for s in range(Sr):
                        q = 64 + 32 * s
                        nc.vector.tensor_copy(
                            out=scst[:, gl * NG + s * 512:
                                     gl * NG + (s + 1) * 512],
                            in_=pS[q:q + 1, 0:512])
                    nc.vector.tensor_copy(
                        out=Sst[:, gl * 640: (gl + 1) * 640]
                            .rearrange("p (i f) -> p i f", i=2, f=320),
                        in_=pS[0:32, 0:1024]
                            .rearrange("p (i f) -> p i f", i=2, f=512)
                            [:, :, 0:320])

                nc.scalar.dma_start(out=scal[sc: sc + 1, :], in_=scst[:])
                nc.scalar.dma_start(out=S_out[sc], in_=Sst[:])

    nc.compile()
    _BUILD_CACHE[key] = nc
    return nc


def _next_pow2(x):
    p = 8
    while p < x:
        p *= 2
    return p


def _host_reference(node_embedding, W1, b1, W2, b2, W3, b3, w_l2, batch,
                    natoms):
    """Pure-numpy fallback (only used for pathological graph layouts)."""
    G = natoms.shape[0]
    inv = 1.0 / natoms.astype(np.float32)
    x = node_embedding[:, 0, :]
    h = x @ W1.T + b1
    h = h / (1.0 + np.exp(-h))
    h = h @ W2.T + b2
    h = h / (1.0 + np.exp(-h))
    ns = (h @ W3.T + b3)[:, 0]
    ok = (batch >= 0) & (batch < G)
    bok = batch[ok]
    iso = np.bincount(bok, weights=ns[ok], minlength=G).astype(np.float32) \
        * inv
    nl2 = np.einsum("nmc,c->nm", node_embedding[:, 4:9, :], w_l2[0])
    aniso = np.stack(
        [np.bincount(bok, weights=nl2[ok, m], minlength=G)
         for m in range(5)], axis=1).astype(np.float32) * inv[:, None]
    dec = np.concatenate([iso[:, None], np.zeros((G, 3), np.float32), aniso],
                         axis=1)
    return (dec @ _CG).reshape(-1, 3, 3).astype(np.float32)


def _chain4_quant(el2, batch):
    """fp8 E4M3 quantization with 4-node error-feedback chains.

    Within each aligned run of 4 nodes, the rounding residual of node k is
    added to node k+1 before its rounding whenever both nodes belong to the
    same graph, so the graph-level segment sum sees ~1 rounding error per
    chain instead of 4."""
    n = el2.shape[0]
    v = el2.reshape(n // 4, 4, ML2)
    b4 = batch.reshape(n // 4, 4)
    out = np.empty((n // 4, 4, ML2), WIRE8)
    carry = np.zeros((n // 4, ML2), np.float32)
    for k in range(4):
        tgt = v[:, k] + carry
        q = tgt.astype(WIRE8)
        out[:, k] = q
        if k < 3:
            same = (b4[:, k] == b4[:, k + 1]).astype(np.float32)[:, None]
            carry = (tgt - q.astype(np.float32)) * same
    return out.reshape(n, ML2)


def kernel(node_embedding, W1, b1, W2, b2, W3, b3, w_l2, batch, natoms):
    node_embedding = np.asarray(node_embedding, dtype=np.float32)
    W1 = np.asarray(W1, dtype=np.float32)
    b1 = np.asarray(b1, dtype=np.float32)
    W2 = np.asarray(W2, dtype=np.float32)
    b2 = np.asarray(b2, dtype=np.float32)
    W3 = np.asarray(W3, dtype=np.float32)
    b3 = np.asarray(b3, dtype=np.float32)
    w_l2 = np.asarray(w_l2, dtype=np.float32)
    batch = np.asarray(batch).astype(np.int64)
    natoms_in = np.asarray(natoms)

    N = node_embedding.shape[0]
    G = natoms_in.shape[0]
    n_sh = (N + N_CORES - 1) // N_CORES
    n_sh = ((n_sh + 3) // 4) * 4       # chain alignment
    n_groups = (n_sh + NG - 1) // NG
    n_pad = n_groups * NG
    n_sc = (n_groups + SCG - 1) // SCG

    if N % 4 != 0:
        return _host_reference(node_embedding, W1, b1, W2, b2, W3, b3,
                               w_l2, batch, natoms_in)

    # per-core shard ranges and per-group base graph ids
    shards = []
    W_need = 8
    for c in range(N_CORES):
        n0 = min(c * n_sh, N)
        n1 = min(n0 + n_sh, N)
        b = batch[n0:n1]
        nreal = n1 - n0
        gbase = np.zeros(n_groups, np.int64)
        for grp in range(n_groups):
            lo = grp * NG
            hi = min(lo + NG, nreal)
            if lo < nreal:
                gbase[grp] = b[lo]
                span = int(b[hi - 1] - b[lo] + 1)
                W_need = max(W_need, span)
        shards.append((n0, n1, b, gbase))
    W = _next_pow2(W_need)
    if (W > 32 or not np.all(batch[:-1] <= batch[1:])
            or batch.min(initial=0) < 0 or batch.max(initial=0) >= G):
        return _host_reference(node_embedding, W1, b1, W2, b2, W3, b3,
                               w_l2, batch, natoms_in)

    nc = _build(n_groups, W, n_sh)

    WIRE16 = ml_dtypes.bfloat16
    w1t = np.ascontiguousarray(W1.T).astype(WIRE16)
    w2t = np.ascontiguousarray(W2.T).astype(WIRE16)
    w3t = np.ascontiguousarray(W3.T).astype(WIRE16)
    b1c = np.ascontiguousarray(b1[:, None])
    b2c = np.ascontiguousarray(b2[:, None])
    iota_c = np.ascontiguousarray(
        np.tile(np.arange(W, dtype=np.float32), (P, 1)))

    # global chain-compensated fp8 of the l=2 block + plain fp8 of x0
    el2q = _chain4_quant(
        node_embedding[:, 4:9, :].reshape(N, ML2), batch)
    x0q = node_embedding[:, 0, :].astype(WIRE8)

    in_maps = []
    for c in range(N_CORES):
        n0, n1, b, gbase = shards[c]
        nreal = n1 - n0
        n_pad_sc = n_sc * SCG * NG
        # x0 wire: [sc, c(128), node] channel-major
        x0T = np.zeros((P, n_pad_sc), WIRE8)
        x0T[:, :nreal] = x0q[n0:n1].T
        x0w = np.ascontiguousarray(
            x0T.reshape(P, n_sc, SCG * NG).transpose(1, 0, 2))
        # el2 wire: node j = g*1024 + d*256 + 2p + i lives at
        # [sc, p, (gl, d, h, i, f320)]
        el2 = np.zeros((n_pad_sc, ML2), WIRE8)
        el2[:nreal] = el2q[n0:n1]
        el2 = el2.reshape(n_sc, SCG, DT, P, 2, 2, 320)
        el2 = np.ascontiguousarray(el2.transpose(0, 3, 1, 2, 5, 4, 6)
                                   .reshape(n_sc, P, SCG * GPB))
        # local graph ids per (g, d, p, i)
        lg = np.full(n_pad, -1.0, np.float32)
        lg[:nreal] = (b - np.repeat(gbase, NG)[:nreal]).astype(np.float32)
        lg_t = np.ascontiguousarray(
            lg.reshape(n_groups, DT, P, 2).transpose(2, 0, 1, 3)
              .reshape(P, n_groups * DT * 2))
        in_maps.append({
            "el2w": el2, "x0w": x0w, "lgid": lg_t, "iota_in": iota_c,
            "w1t": w1t, "w2t": w2t, "w3t": w3t, "b1c": b1c, "b2c": b2c,
        })

    res = bass_utils.run_bass_kernel_spmd(nc, in_maps,
                                          core_ids=list(range(N_CORES)))

    # ---- host epilogue ----
    inv = (1.0 / natoms_in.astype(np.float32)).astype(np.float32)
    node_scalar = np.empty(N, np.float32)
    Sfull = np.zeros((G + 32, ML2), np.float32)
    for c in range(N_CORES):
        n0, n1, _, gbase = shards[c]
        nreal = n1 - n0
        sc = res.results[c]["scal"].reshape(-1)[:nreal]
        node_scalar[n0:n1] = sc
        Sc = res.results[c]["S_out"]        # (n_sc, 32, SCG*640) f32
        for grp in range(n_groups):
            if grp * NG < nreal:
                gb = int(gbase[grp])
                blk = Sc[grp // SCG][:, (grp % SCG) * 640:
                                     (grp % SCG + 1) * 640]
                Sfull[gb:gb + W, :] += blk[0:W]
    iso = np.bincount(batch, weights=node_scalar + b3[0], minlength=G)
    iso = iso.astype(np.float32) * inv
    aniso = (Sfull[:G].reshape(G, 5, P) @ w_l2[0]).astype(np.float32)
    aniso *= inv[:, None]
    dec = np.concatenate([iso[:, None], np.zeros((G, 3), np.float32), aniso],
                         axis=1)
    return (dec @ _CG).reshape(-1, 3, 3).astype(np.float32)


# revision 17
# speedup vs baseline: 1.2104x; 1.2104x over previous
"""CartBasisStressHead kernel for Trainium2 (8 NeuronCores, SPMD data-parallel).

Strategy
--------
Only 6 of the 9 m-rows of node_embedding are used: row 0 feeds a SiLU MLP
(per-node scalar), rows 4:9 feed a per-channel contraction (l=2 branch).
Nodes are sharded contiguously across 8 cores; segment sums are linear, so
the host adds per-shard partials.

The kernel is HBM-bound, so everything streamed is wired in fp8 (E4M3):
  * l=2 data uses a chain-of-4 compensated quantization (each node's rounding
    residual is folded into the next node of the same graph before rounding),
    cutting the segment-sum quantization error ~2x below plain fp8 rounding.
  * The segment sum itself runs on the PE in DoubleRow fp8 perf mode:
    256 nodes per pass (128 partitions x 2 pair lanes), with a 0/1 indicator
    matrix A[node, local_graph] as the stationary operand. DoubleRow only
    supports PSUM quadrant 0, so the two 320-column halves of the l=2
    features accumulate into two separate PSUM tiles.
  * The MLP runs x0 (fp8) against bf16 weights; per-node scalars come from a
    1-wide W3 matmul packed into spare PE column bands.

Inputs stream as 5-group superchunks, each split into a 1-group head DMA
(fast pipeline ramp) plus a 4-group rest DMA (~2.6 MB, near-peak HBM
efficiency); outputs stage in SBUF and store once per superchunk. Per-group
PE issue order interleaves the l=2 passes around the MLP layers so the
in-order PE queue never waits on the activation engine.

Host epilogue: scatter-add of per-group segment partials, contraction with
w_l2, bincount of per-node scalars, and the tiny (G,9)@(9,9) basis change.
"""

import sys

if "/opt/trn_rl_repo" not in sys.path:
    sys.path.insert(0, "/opt/trn_rl_repo")

import numpy as np
import ml_dtypes

import concourse.bacc as bacc
import concourse.tile as tile
from concourse import mybir
from concourse import bass_utils

_S2 = 2.0 ** -0.5
_S3 = 3.0 ** -0.5
_S6 = 6.0 ** -0.5
_CG = np.array([
    [_S3, 0, 0, 0, _S3, 0, 0, 0, _S3],
    [0, 0, 0, 0, 0, _S2, 0, -_S2, 0],
    [0, 0, -_S2, 0, 0, 0, _S2, 0, 0],
    [0, _S2, 0, -_S2, 0, 0, 0, 0, 0],
    [0, 0, _S2, 0, 0, 0, _S2, 0, 0],
    [0, 0, 0, 0, 0, _S2, 0, _S2, 0],
    [-_S6, 0, 0, 0, 2 * _S6, 0, 0, 0, -_S6],
    [0, _S2, 0, _S2, 0, 0, 0, 0, 0],
    [-_S2, 0, 0, 0, 0, 0, 0, 0, _S2],
], dtype=np.float32)  # (9, 9)

N_CORES = 8
P = 128          # SBUF partitions
NG = 1024        # nodes per group (one PSUM accumulation span)
DT = 4           # 256-node dtiles per group (DoubleRow pairs 2 nodes/lane)
ML2 = 640        # l=2 values per node (5 m-rows x 128 channels)
SCG = 5          # groups per superchunk (one input DMA / output store)
GPB = 5120       # el2 bytes per partition per group

F32 = mybir.dt.float32
BF16 = mybir.dt.bfloat16
F8 = mybir.dt.float8e4
DR = mybir.MatmulPerfMode.DoubleRow
WIRE8 = ml_dtypes.float8_e4m3

_BUILD_CACHE = {}


def _build(n_groups, W, n_real):
    key = (n_groups, W, n_real)
    if key in _BUILD_CACHE:
        return _BUILD_CACHE[key]

    n_sc = (n_groups + SCG - 1) // SCG
    T2 = n_groups * DT * 2

    nc = bacc.Bacc("TRN2", target_bir_lowering=False, debug=False,
                   num_devices=N_CORES)

    # inputs (host pre-packed; see kernel() for layouts)
    el2w = nc.dram_tensor("el2w", (n_sc, P, SCG * GPB), F8,
                          kind="ExternalInput").ap()
    x0w = nc.dram_tensor("x0w", (n_sc, P, SCG * NG), F8,
                         kind="ExternalInput").ap()
    lgid = nc.dram_tensor("lgid", (P, T2), F32, kind="ExternalInput").ap()
    iota_in = nc.dram_tensor("iota_in", (P, W), F32, kind="ExternalInput").ap()
    w1t = nc.dram_tensor("w1t", (P, P), BF16, kind="ExternalInput").ap()
    w2t = nc.dram_tensor("w2t", (P, P), BF16, kind="ExternalInput").ap()
    w3t = nc.dram_tensor("w3t", (P, 2), BF16, kind="ExternalInput").ap()
    b1 = nc.dram_tensor("b1c", (P, 1), F32, kind="ExternalInput").ap()
    b2 = nc.dram_tensor("b2c", (P, 1), F32, kind="ExternalInput").ap()
    # output: rows 0:W = l=2 segment partials (halves at cols i*512+0:320),
    # row 32 = per-node scalars (chunk s at cols s*512:...)
    S_out = nc.dram_tensor("S_out", (n_sc, 34, SCG * NG), BF16,
                           kind="ExternalOutput").ap()

    silu = mybir.ActivationFunctionType.Silu
    eq = mybir.AluOpType.is_equal

    with tile.TileContext(nc) as tc:
        with (
            tc.tile_pool(name="const", bufs=1) as cpool,
            tc.tile_pool(name="el2h", bufs=3) as el2hp,
            tc.tile_pool(name="el2r", bufs=3) as el2rp,
            tc.tile_pool(name="x0p", bufs=3) as x0p,
            tc.tile_pool(name="hp", bufs=4) as hp,
            tc.tile_pool(name="stp", bufs=2) as stp,
            tc.tile_pool(name="ph1", bufs=1, space="PSUM") as ph1p,
            tc.tile_pool(name="ph2", bufs=1, space="PSUM") as ph2p,
            tc.tile_pool(name="pS", bufs=2, space="PSUM") as pSp,
        ):
            w1s = cpool.tile([P, P], BF16)
            w2s = cpool.tile([P, P], BF16)
            w3s = cpool.tile([P, 2], BF16)
            b1s = cpool.tile([P, 1], F32)
            b2s = cpool.tile([P, 1], F32)
            iotas = cpool.tile([P, W], F32)
            lgids = cpool.tile([P, T2], F32)
            # lgid/iota ride the sync queue ahead of the bulk el2 streams so
            # the indicator build never waits behind megabytes of input
            nc.sync.dma_start(out=lgids[:], in_=lgid)
            nc.sync.dma_start(out=iotas[:], in_=iota_in)
            nc.scalar.dma_start(out=w1s[:], in_=w1t)
            nc.scalar.dma_start(out=w2s[:], in_=w2t)
            nc.scalar.dma_start(out=w3s[:], in_=w3t)
            nc.scalar.dma_start(out=b1s[:], in_=b1)
            nc.scalar.dma_start(out=b2s[:], in_=b2)

            # all indicator pair-matrices up front (one DVE instruction):
            # Aall[p, (g,d,i), w] = (iota[w] == lgid[p, (g,d,i)])
            Aall = cpool.tile([P, T2 * W], F8)
            nc.vector.tensor_tensor(
                out=Aall[:].rearrange("p (t w) -> p t w", t=T2, w=W),
                in0=iotas[:].unsqueeze(1).to_broadcast([P, T2, W]),
                in1=lgids[:].unsqueeze(2).to_broadcast([P, T2, W]),
                op=eq)

            for sc in range(n_sc):
                sc_g0 = sc * SCG
                sc_ng = min(SCG, n_groups - sc_g0)
                last_g = sc_g0 + sc_ng - 1
                lg_real = min(NG, n_real - last_g * NG)
                dt_last = (lg_real + 255) // 256
                sr_last = (lg_real + 511) // 512

                # all inputs ride the sync queue (the scalar-queue HWDGE ring
                # is strictly deprioritized behind sync); per superchunk the
                # order is x0 -> head group -> rest so compute ramps early
                x0c = x0p.tile([P, SCG * NG], F8, tag="x0c")
                xext = (sc_ng - 1) * NG + sr_last * 512
                nc.sync.dma_start(out=x0c[:, :xext],
                                  in_=x0w[sc][:, :xext])
                el2h = el2hp.tile([P, GPB], F8, tag="el2h")
                nc.sync.dma_start(
                    out=el2h[:, :GPB if sc_ng > 1 else dt_last * 1280],
                    in_=el2w[sc][:, :GPB if sc_ng > 1 else dt_last * 1280])
                el2r = el2rp.tile([P, (SCG - 1) * GPB], F8, tag="el2r")
                if sc_ng > 1:
                    ext = (sc_ng - 2) * GPB + dt_last * 1280
                    nc.sync.dma_start(out=el2r[:, :ext],
                                      in_=el2w[sc][:, GPB: GPB + ext])

                Sst = stp.tile([34, SCG * NG], BF16, tag="Sst")

                for gl in range(sc_ng):
                    g = sc_g0 + gl
                    grp_real = min(NG, n_real - g * NG)
                    Sr = (grp_real + 511) // 512
                    Dr = (grp_real + 255) // 256
                    if gl == 0:
                        esrc = el2h
                        ebase = 0
                    else:
                        esrc = el2r
                        ebase = (gl - 1) * GPB

                    # one 2-bank PSUM tile per group:
                    #   [0:W, 0:320]    l=2 feature half 0 (bank A)
                    #   [0:W, 512:832]  l=2 feature half 1 (bank B)
                    #   [64:65, 0:512]  W3 scalars chunk 0
                    #   [96:97, 0:512]  W3 scalars chunk 1
                    pS = pSp.tile([P, 1024], F32, tag="pS")

                    def l2mm(d):
                        t2i = (g * DT + d) * 2
                        Ad = Aall[:, t2i * W: (t2i + 2) * W] \
                            .rearrange("p (i w) -> p i w", i=2, w=W)
                        base = ebase + d * 1280
                        nc.tensor.matmul(
                            pS[0:W, 0:320], Ad,
                            esrc[:, base: base + 640]
                                .rearrange("p (i f) -> p i f", i=2, f=320),
                            start=(d == 0), stop=(d == Dr - 1),
                            perf_mode=DR, tile_position=(0, 0))
                        nc.tensor.matmul(
                            pS[0:W, 512:832], Ad,
                            esrc[:, base + 640: base + 1280]
                                .rearrange("p (i f) -> p i f", i=2, f=320),
                            start=(d == 0), stop=(d == Dr - 1),
                            perf_mode=DR, tile_position=(0, 0))

                    # ---- MLP layer 1 (both chunks into one 2-bank tile) ----
                    h1p = ph1p.tile([P, 1024], F32, tag="h1p")
                    for s in range(Sr):
                        nsl = slice(gl * NG + s * 512, gl * NG + (s + 1) * 512)
                        nc.tensor.matmul(h1p[:, s * 512:(s + 1) * 512],
                                         w1s[:], x0c[:, nsl],
                                         start=True, stop=True)
                    h1s = hp.tile([P, 1024], BF16, tag="h1s")
                    nc.scalar.activation(h1s[:, :Sr * 512],
                                         h1p[:, :Sr * 512], silu, bias=b1s[:])

                    # ---- l=2 segment sum, first half of the dtiles ----
                    for d in range((Dr + 1) // 2):
                        l2mm(d)

                    # ---- MLP layer 2 (h1 ready by now; one batched silu) ----
                    h2p = ph2p.tile([P, 1024], F32, tag="h2p")
                    for s in range(Sr):
                        nc.tensor.matmul(h2p[:, s * 512:(s + 1) * 512],
                                         w2s[:], h1s[:, s * 512:(s + 1) * 512],
                                         start=True, stop=True)
                    h2s = hp.tile([P, 1024], BF16, tag="h2s")
                    nc.scalar.activation(h2s[:, :Sr * 512],
                                         h2p[:, :Sr * 512], silu, bias=b2s[:])

                    # ---- l=2 segment sum, second half ----
                    for d in range((Dr + 1) // 2, Dr):
                        l2mm(d)

                    # ---- per-node scalar: w3 (duplicated 2-wide) @ h2 ----
                    # chunk s lands at pS[32:34, s*512:(s+1)*512]
                    for s in range(Sr):
                        nc.tensor.matmul(pS[32:34, s * 512:(s + 1) * 512],
                                         w3s[:],
                                         h2s[:, s * 512:(s + 1) * 512],
                                         start=True, stop=True,
                                         tile_position=(0, 32))

                    # ---- one staging copy per group (S rows + scalars) ----
                    nc.vector.tensor_copy(
                        out=Sst[:, gl * NG: (gl + 1) * NG],
                        in_=pS[0:34, 0:1024])

                nc.gpsimd.dma_start(out=S_out[sc], in_=Sst[:])

    nc.compile()
    _BUILD_CACHE[key] = nc
    return nc


def _next_pow2(x):
    p = 8
    while p < x:
        p *= 2
    return p


def _host_reference(node_embedding, W1, b1, W2, b2, W3, b3, w_l2, batch,
                    natoms):
    """Pure-numpy fallback (only used for pathological graph layouts)."""
    G = natoms.shape[0]
    inv = 1.0 / natoms.astype(np.float32)
    x = node_embedding[:, 0, :]
    h = x @ W1.T + b1
    h = h / (1.0 + np.exp(-h))
    h = h @ W2.T + b2
    h = h / (1.0 + np.exp(-h))
    ns = (h @ W3.T + b3)[:, 0]
    ok = (batch >= 0) & (batch < G)
    bok = batch[ok]
    iso = np.bincount(bok, weights=ns[ok], minlength=G).astype(np.float32) \
        * inv
    nl2 = np.einsum("nmc,c->nm", node_embedding[:, 4:9, :], w_l2[0])
    aniso = np.stack(
        [np.bincount(bok, weights=nl2[ok, m], minlength=G)
         for m in range(5)], axis=1).astype(np.float32) * inv[:, None]
    dec = np.concatenate([iso[:, None], np.zeros((G, 3), np.float32), aniso],
                         axis=1)
    return (dec @ _CG).reshape(-1, 3, 3).astype(np.float32)


def _chain4_quant(el2, batch):
    """fp8 E4M3 quantization with 4-node error-feedback chains.

    Within each aligned run of 4 nodes, the rounding residual of node k is
    added to node k+1 before its rounding whenever both nodes belong to the
    same graph, so the graph-level segment sum sees ~1 rounding error per
    chain instead of 4."""
    n = el2.shape[0]
    v = el2.reshape(n // 4, 4, ML2)
    b4 = batch.reshape(n // 4, 4)
    out = np.empty((n // 4, 4, ML2), WIRE8)
    carry = np.zeros((n // 4, ML2), np.float32)
    for k in range(4):
        tgt = v[:, k] + carry
        q = tgt.astype(WIRE8)
        out[:, k] = q
        if k < 3:
            same = (b4[:, k] == b4[:, k + 1]).astype(np.float32)[:, None]
            carry = (tgt - q.astype(np.float32)) * same
    return out.reshape(n, ML2)


def kernel(node_embedding, W1, b1, W2, b2, W3, b3, w_l2, batch, natoms):
    node_embedding = np.asarray(node_embedding, dtype=np.float32)
    W1 = np.asarray(W1, dtype=np.float32)
    b1 = np.asarray(b1, dtype=np.float32)
    W2 = np.asarray(W2, dtype=np.float32)
    b2 = np.asarray(b2, dtype=np.float32)
    W3 = np.asarray(W3, dtype=np.float32)
    b3 = np.asarray(b3, dtype=np.float32)
    w_l2 = np.asarray(w_l2, dtype=np.float32)
    batch = np.asarray(batch).astype(np.int64)
    natoms_in = np.asarray(natoms)

    N = node_embedding.shape[0]
    G = natoms_in.shape[0]
    n_sh = (N + N_CORES - 1) // N_CORES
    n_sh = ((n_sh + 3) // 4) * 4       # chain alignment
    n_groups = (n_sh + NG - 1) // NG
    n_pad = n_groups * NG
    n_sc = (n_groups + SCG - 1) // SCG

    if N % 4 != 0:
        return _host_reference(node_embedding, W1, b1, W2, b2, W3, b3,
                               w_l2, batch, natoms_in)

    # per-core shard ranges and per-group base graph ids
    shards = []
    W_need = 8
    for c in range(N_CORES):
        n0 = min(c * n_sh, N)
        n1 = min(n0 + n_sh, N)
        b = batch[n0:n1]
        nreal = n1 - n0
        gbase = np.zeros(n_groups, np.int64)
        for grp in range(n_groups):
            lo = grp * NG
            hi = min(lo + NG, nreal)
            if lo < nreal:
                gbase[grp] = b[lo]
                span = int(b[hi - 1] - b[lo] + 1)
                W_need = max(W_need, span)
        shards.append((n0, n1, b, gbase))
    W = _next_pow2(W_need)
    if (W > 32 or not np.all(batch[:-1] <= batch[1:])
            or batch.min(initial=0) < 0 or batch.max(initial=0) >= G):
        return _host_reference(node_embedding, W1, b1, W2, b2, W3, b3,
                               w_l2, batch, natoms_in)

    nc = _build(n_groups, W, n_sh)

    WIRE16 = ml_dtypes.bfloat16
    w1t = np.ascontiguousarray(W1.T).astype(WIRE16)
    w2t = np.ascontiguousarray(W2.T).astype(WIRE16)
    w3t = np.ascontiguousarray(np.repeat(W3.T, 2, axis=1)).astype(WIRE16)
    b1c = np.ascontiguousarray(b1[:, None])
    b2c = np.ascontiguousarray(b2[:, None])
    iota_c = np.ascontiguousarray(
        np.tile(np.arange(W, dtype=np.float32), (P, 1)))

    # global chain-compensated fp8 of the l=2 block + plain fp8 of x0
    el2q = _chain4_quant(
        node_embedding[:, 4:9, :].reshape(N, ML2), batch)
    x0q = node_embedding[:, 0, :].astype(WIRE8)

    in_maps = []
    for c in range(N_CORES):
        n0, n1, b, gbase = shards[c]
        nreal = n1 - n0
        n_pad_sc = n_sc * SCG * NG
        # x0 wire: [sc, c(128), node] channel-major
        x0T = np.zeros((P, n_pad_sc), WIRE8)
        x0T[:, :nreal] = x0q[n0:n1].T
        x0w = np.ascontiguousarray(
            x0T.reshape(P, n_sc, SCG * NG).transpose(1, 0, 2))
        # el2 wire: node j = g*1024 + d*256 + 2p + i lives at
        # [sc, p, (gl, d, h, i, f320)]
        el2 = np.zeros((n_pad_sc, ML2), WIRE8)
        el2[:nreal] = el2q[n0:n1]
        el2 = el2.reshape(n_sc, SCG, DT, P, 2, 2, 320)
        el2 = np.ascontiguousarray(el2.transpose(0, 3, 1, 2, 5, 4, 6)
                                   .reshape(n_sc, P, SCG * GPB))
        # local graph ids per (g, d, p, i)
        lg = np.full(n_pad, -1.0, np.float32)
        lg[:nreal] = (b - np.repeat(gbase, NG)[:nreal]).astype(np.float32)
        lg_t = np.ascontiguousarray(
            lg.reshape(n_groups, DT, P, 2).transpose(2, 0, 1, 3)
              .reshape(P, n_groups * DT * 2))
        in_maps.append({
            "el2w": el2, "x0w": x0w, "lgid": lg_t, "iota_in": iota_c,
            "w1t": w1t, "w2t": w2t, "w3t": w3t, "b1c": b1c, "b2c": b2c,
        })

    res = bass_utils.run_bass_kernel_spmd(nc, in_maps,
                                          core_ids=list(range(N_CORES)))

    # ---- host epilogue ----
    inv = (1.0 / natoms_in.astype(np.float32)).astype(np.float32)
    node_scalar = np.empty(N, np.float32)
    Sfull = np.zeros((G + 32, ML2), np.float32)
    for c in range(N_CORES):
        n0, n1, _, gbase = shards[c]
        nreal = n1 - n0
        Sc = np.asarray(res.results[c]["S_out"]).astype(np.float32)
        # (n_sc, 34, SCG*1024): row 32 = per-node scalars, rows 0:W = S
        node_scalar[n0:n1] = Sc[:, 32, :].reshape(-1)[:nreal]
        for grp in range(n_groups):
            if grp * NG < nreal:
                gb = int(gbase[grp])
                blk = Sc[grp // SCG][:, (grp % SCG) * NG:
                                     (grp % SCG + 1) * NG]
                Sfull[gb:gb + W, 0:320] += blk[0:W, 0:320]
                Sfull[gb:gb + W, 320:640] += blk[0:W, 512:832]
    iso = np.bincount(batch, weights=node_scalar + b3[0], minlength=G)
    iso = iso.astype(np.float32) * inv
    aniso = (Sfull[:G].reshape(G, 5, P) @ w_l2[0]).astype(np.float32)
    aniso *= inv[:, None]
    dec = np.concatenate([iso[:, None], np.zeros((G, 3), np.float32), aniso],
                         axis=1)
    return (dec @ _CG).reshape(-1, 3, 3).astype(np.float32)


# revision 19
# speedup vs baseline: 1.2856x; 1.0620x over previous
"""CartBasisStressHead kernel for Trainium2 (8 NeuronCores, SPMD data-parallel).

Strategy
--------
Only 6 of the 9 m-rows of node_embedding are used: row 0 feeds a SiLU MLP
(per-node scalar), rows 4:9 feed a per-channel contraction (l=2 branch).
Nodes are sharded contiguously across 8 cores; segment sums are linear, so
the host adds per-shard partials.

The kernel is HBM-bound, so everything streamed is wired in fp8 (E4M3):
  * l=2 data uses a chain-of-4 compensated quantization (each node's rounding
    residual is folded into the next node of the same graph before rounding),
    cutting the segment-sum quantization error ~2x below plain fp8 rounding.
  * The segment sum itself runs on the PE in DoubleRow fp8 perf mode:
    256 nodes per pass (128 partitions x 2 pair lanes), with a 0/1 indicator
    matrix A[node, local_graph] as the stationary operand. DoubleRow only
    supports PSUM quadrant 0, so the two 320-column halves of the l=2
    features accumulate into two separate PSUM tiles.
  * The MLP runs x0 (fp8) against bf16 weights; per-node scalars come from a
    1-wide W3 matmul packed into spare PE column bands.

Inputs stream as 5-group superchunks, each split into a 1-group head DMA
(fast pipeline ramp) plus a 4-group rest DMA (~2.6 MB, near-peak HBM
efficiency); outputs stage in SBUF and store once per superchunk. Per-group
PE issue order interleaves the l=2 passes around the MLP layers so the
in-order PE queue never waits on the activation engine.

Host epilogue: scatter-add of per-group segment partials, contraction with
w_l2, bincount of per-node scalars, and the tiny (G,9)@(9,9) basis change.
"""

import sys

if "/opt/trn_rl_repo" not in sys.path:
    sys.path.insert(0, "/opt/trn_rl_repo")

import numpy as np
import ml_dtypes

import concourse.bacc as bacc
import concourse.tile as tile
from concourse import mybir
from concourse import bass_utils

_S2 = 2.0 ** -0.5
_S3 = 3.0 ** -0.5
_S6 = 6.0 ** -0.5
_CG = np.array([
    [_S3, 0, 0, 0, _S3, 0, 0, 0, _S3],
    [0, 0, 0, 0, 0, _S2, 0, -_S2, 0],
    [0, 0, -_S2, 0, 0, 0, _S2, 0, 0],
    [0, _S2, 0, -_S2, 0, 0, 0, 0, 0],
    [0, 0, _S2, 0, 0, 0, _S2, 0, 0],
    [0, 0, 0, 0, 0, _S2, 0, _S2, 0],
    [-_S6, 0, 0, 0, 2 * _S6, 0, 0, 0, -_S6],
    [0, _S2, 0, _S2, 0, 0, 0, 0, 0],
    [-_S2, 0, 0, 0, 0, 0, 0, 0, _S2],
], dtype=np.float32)  # (9, 9)

N_CORES = 8
P = 128          # SBUF partitions
NG = 1024        # nodes per group (one PSUM accumulation span)
DT = 4           # 256-node dtiles per group (DoubleRow pairs 2 nodes/lane)
ML2 = 640        # l=2 values per node (5 m-rows x 128 channels)
SCG = 5          # groups per superchunk (one input DMA / output store)
GPB = 5120       # el2 bytes per partition per group

F32 = mybir.dt.float32
BF16 = mybir.dt.bfloat16
F8 = mybir.dt.float8e4
DR = mybir.MatmulPerfMode.DoubleRow
WIRE8 = ml_dtypes.float8_e4m3

_BUILD_CACHE = {}


def _build(n_groups, W, n_real):
    key = (n_groups, W, n_real)
    if key in _BUILD_CACHE:
        return _BUILD_CACHE[key]

    n_sc = (n_groups + SCG - 1) // SCG
    T2 = n_groups * DT * 2

    nc = bacc.Bacc("TRN2", target_bir_lowering=False, debug=False,
                   num_devices=N_CORES)

    # inputs (host pre-packed; see kernel() for layouts)
    el2w = nc.dram_tensor("el2w", (n_sc, P, SCG * GPB), F8,
                          kind="ExternalInput").ap()
    x0w = nc.dram_tensor("x0w", (n_sc, P, SCG * NG), F8,
                         kind="ExternalInput").ap()
    lgid = nc.dram_tensor("lgid", (P, T2), F32, kind="ExternalInput").ap()
    iota_in = nc.dram_tensor("iota_in", (P, W), F32, kind="ExternalInput").ap()
    w1t = nc.dram_tensor("w1t", (P, P), BF16, kind="ExternalInput").ap()
    w2t = nc.dram_tensor("w2t", (P, P), BF16, kind="ExternalInput").ap()
    w3t = nc.dram_tensor("w3t", (P, 2), BF16, kind="ExternalInput").ap()
    b1 = nc.dram_tensor("b1c", (P, 1), F32, kind="ExternalInput").ap()
    b2 = nc.dram_tensor("b2c", (P, 1), F32, kind="ExternalInput").ap()
    # output: rows 0:W = l=2 segment partials (halves at cols i*512+0:320),
    # row 32 = per-node scalars (chunk s at cols s*512:...)
    S_out = nc.dram_tensor("S_out", (n_sc, 34, SCG * NG), BF16,
                           kind="ExternalOutput").ap()

    silu = mybir.ActivationFunctionType.Silu
    eq = mybir.AluOpType.is_equal

    with tile.TileContext(nc) as tc:
        with (
            tc.tile_pool(name="const", bufs=1) as cpool,
            tc.tile_pool(name="el2h", bufs=3) as el2hp,
            tc.tile_pool(name="el2r", bufs=3) as el2rp,
            tc.tile_pool(name="x0p", bufs=3) as x0p,
            tc.tile_pool(name="hp", bufs=4) as hp,
            tc.tile_pool(name="stp", bufs=2) as stp,
            tc.tile_pool(name="ph1", bufs=1, space="PSUM") as ph1p,
            tc.tile_pool(name="ph2", bufs=1, space="PSUM") as ph2p,
            tc.tile_pool(name="pS", bufs=2, space="PSUM") as pSp,
        ):
            w1s = cpool.tile([P, P], BF16)
            w2s = cpool.tile([P, P], BF16)
            w3s = cpool.tile([P, 2], BF16)
            b1s = cpool.tile([P, 1], F32)
            b2s = cpool.tile([P, 1], F32)
            iotas = cpool.tile([P, W], F32)
            lgids = cpool.tile([P, T2], F32)
            # every input rides the sync queue (the scalar-ring HWDGE is
            # strictly deprioritized behind sync); consts first, then all
            # superchunks [x0, el2 head, el2 rest] — the sync engine's
            # in-order buffer waits provide natural flow control
            nc.sync.dma_start(out=w1s[:], in_=w1t)
            nc.sync.dma_start(out=w2s[:], in_=w2t)
            nc.sync.dma_start(out=w3s[:], in_=w3t)
            nc.sync.dma_start(out=b1s[:], in_=b1)
            nc.sync.dma_start(out=b2s[:], in_=b2)
            nc.sync.dma_start(out=lgids[:], in_=lgid)
            nc.sync.dma_start(out=iotas[:], in_=iota_in)

            # all indicator pair-matrices up front (one DVE instruction):
            # Aall[p, (g,d,i), w] = (iota[w] == lgid[p, (g,d,i)])
            Aall = cpool.tile([P, T2 * W], F8)
            nc.vector.tensor_tensor(
                out=Aall[:].rearrange("p (t w) -> p t w", t=T2, w=W),
                in0=iotas[:].unsqueeze(1).to_broadcast([P, T2, W]),
                in1=lgids[:].unsqueeze(2).to_broadcast([P, T2, W]),
                op=eq)

            x0cs, el2hs, el2rs = [], [], []
            for sc in range(n_sc):
                sc_g0 = sc * SCG
                sc_ng = min(SCG, n_groups - sc_g0)
                last_g = sc_g0 + sc_ng - 1
                lg_real = min(NG, n_real - last_g * NG)
                dt_last = (lg_real + 255) // 256
                sr_last = (lg_real + 511) // 512
                x0c = x0p.tile([P, SCG * NG], F8, tag="x0c")
                xext = (sc_ng - 1) * NG + sr_last * 512
                nc.sync.dma_start(out=x0c[:, :xext], in_=x0w[sc][:, :xext])
                el2h = el2hp.tile([P, GPB], F8, tag="el2h")
                hext = GPB if sc_ng > 1 else dt_last * 1280
                nc.sync.dma_start(out=el2h[:, :hext],
                                  in_=el2w[sc][:, :hext])
                el2r = el2rp.tile([P, (SCG - 1) * GPB], F8, tag="el2r")
                if sc_ng > 1:
                    ext = (sc_ng - 2) * GPB + dt_last * 1280
                    nc.sync.dma_start(out=el2r[:, :ext],
                                      in_=el2w[sc][:, GPB: GPB + ext])
                x0cs.append(x0c)
                el2hs.append(el2h)
                el2rs.append(el2r)

            # software-pipelined compute: iteration `it` issues
            #   L1(it) -> L2(it-1) -> l2(it) -> W3(it-1) -> copy(it-1)
            # so every matmul's cross-engine inputs are a full group old and
            # the in-order PE queue never stalls on the activation engine.
            # Matmuls sharing a stationary are paired; the second reuses the
            # PE-resident weights (ldweights=False).
            Ssts = {}
            h1map, h2map, pSmap = {}, {}, {}
            for it in range(n_groups + 1):
                if it < n_groups:
                    g = it
                    sc = g // SCG
                    gl = g % SCG
                    grp_real = min(NG, n_real - g * NG)
                    Sr = (grp_real + 511) // 512
                    if gl == 0:
                        Sst = stp.tile([34, SCG * NG], BF16, tag="Sst")
                        Ssts[sc] = Sst

                    # ---- MLP layer 1 of group g ----
                    h1p = ph1p.tile([P, 1024], F32, tag="h1p")
                    for s in range(Sr):
                        nsl = slice(gl * NG + s * 512, gl * NG + (s + 1) * 512)
                        mm = nc.tensor.matmul(h1p[:, s * 512:(s + 1) * 512],
                                              w1s[:], x0cs[sc][:, nsl],
                                              start=True, stop=True)
                        if s > 0:
                            mm.ldweights = False
                    h1s = hp.tile([P, 1024], BF16, tag="h1s")
                    nc.scalar.activation(h1s[:, :Sr * 512],
                                         h1p[:, :Sr * 512], silu, bias=b1s[:])
                    h1map[g] = (h1s, Sr)

                if it >= 1:
                    # ---- MLP layer 2 of group it-1 ----
                    gp = it - 1
                    h1s_p, Sr_p = h1map.pop(gp)
                    h2p = ph2p.tile([P, 1024], F32, tag="h2p")
                    for s in range(Sr_p):
                        mm = nc.tensor.matmul(
                            h2p[:, s * 512:(s + 1) * 512], w2s[:],
                            h1s_p[:, s * 512:(s + 1) * 512],
                            start=True, stop=True)
                        if s > 0:
                            mm.ldweights = False
                    h2s = hp.tile([P, 1024], BF16, tag="h2s")
                    nc.scalar.activation(h2s[:, :Sr_p * 512],
                                         h2p[:, :Sr_p * 512], silu,
                                         bias=b2s[:])
                    h2map[gp] = (h2s, Sr_p)

                if it < n_groups:
                    # ---- l=2 segment sum of group g (DoubleRow fp8) ----
                    # one 2-bank PSUM tile per group:
                    #   [0:W, 0:320]    l=2 feature half 0 (bank A)
                    #   [0:W, 512:832]  l=2 feature half 1 (bank B)
                    #   [32:34, 0:512]  W3 scalars chunk 0 (dup rows)
                    #   [32:34, 512:1024] W3 scalars chunk 1
                    Dr = (grp_real + 255) // 256
                    pS = pSp.tile([P, 1024], F32, tag="pS")
                    if gl == 0:
                        esrc = el2hs[sc]
                        ebase = 0
                    else:
                        esrc = el2rs[sc]
                        ebase = (gl - 1) * GPB
                    for d in range(Dr):
                        t2i = (g * DT + d) * 2
                        Ad = Aall[:, t2i * W: (t2i + 2) * W] \
                            .rearrange("p (i w) -> p i w", i=2, w=W)
                        base = ebase + d * 1280
                        nc.tensor.matmul(
                            pS[0:W, 0:320], Ad,
                            esrc[:, base: base + 640]
                                .rearrange("p (i f) -> p i f", i=2, f=320),
                            start=(d == 0), stop=(d == Dr - 1),
                            perf_mode=DR, tile_position=(0, 0))
                        mm = nc.tensor.matmul(
                            pS[0:W, 512:832], Ad,
                            esrc[:, base + 640: base + 1280]
                                .rearrange("p (i f) -> p i f", i=2, f=320),
                            start=(d == 0), stop=(d == Dr - 1),
                            perf_mode=DR, tile_position=(0, 0))
                        mm.ldweights = False
                    pSmap[g] = pS

                if it >= 1:
                    # ---- per-node scalars + staging copy of group it-1 ----
                    gp = it - 1
                    h2s_p, Sr_p = h2map.pop(gp)
                    pS_p = pSmap.pop(gp)
                    for s in range(Sr_p):
                        mm = nc.tensor.matmul(
                            pS_p[32:34, s * 512:(s + 1) * 512], w3s[:],
                            h2s_p[:, s * 512:(s + 1) * 512],
                            start=True, stop=True, tile_position=(0, 32))
                        if s > 0:
                            mm.ldweights = False
                    scp = gp // SCG
                    glp = gp % SCG
                    nc.vector.tensor_copy(
                        out=Ssts[scp][:, glp * NG: (glp + 1) * NG],
                        in_=pS_p[0:34, 0:1024])
                    if gp == n_groups - 1 or glp == SCG - 1:
                        nc.gpsimd.dma_start(out=S_out[scp],
                                            in_=Ssts.pop(scp)[:])

    nc.compile()
    _BUILD_CACHE[key] = nc
    return nc


def _next_pow2(x):
    p = 8
    while p < x:
        p *= 2
    return p


def _host_reference(node_embedding, W1, b1, W2, b2, W3, b3, w_l2, batch,
                    natoms):
    """Pure-numpy fallback (only used for pathological graph layouts)."""
    G = natoms.shape[0]
    inv = 1.0 / natoms.astype(np.float32)
    x = node_embedding[:, 0, :]
    h = x @ W1.T + b1
    h = h / (1.0 + np.exp(-h))
    h = h @ W2.T + b2
    h = h / (1.0 + np.exp(-h))
    ns = (h @ W3.T + b3)[:, 0]
    ok = (batch >= 0) & (batch < G)
    bok = batch[ok]
    iso = np.bincount(bok, weights=ns[ok], minlength=G).astype(np.float32) \
        * inv
    nl2 = np.einsum("nmc,c->nm", node_embedding[:, 4:9, :], w_l2[0])
    aniso = np.stack(
        [np.bincount(bok, weights=nl2[ok, m], minlength=G)
         for m in range(5)], axis=1).astype(np.float32) * inv[:, None]
    dec = np.concatenate([iso[:, None], np.zeros((G, 3), np.float32), aniso],
                         axis=1)
    return (dec @ _CG).reshape(-1, 3, 3).astype(np.float32)


def _chain4_quant(el2, batch):
    """fp8 E4M3 quantization with 4-node error-feedback chains.

    Within each aligned run of 4 nodes, the rounding residual of node k is
    added to node k+1 before its rounding whenever both nodes belong to the
    same graph, so the graph-level segment sum sees ~1 rounding error per
    chain instead of 4."""
    n = el2.shape[0]
    v = el2.reshape(n // 4, 4, ML2)
    b4 = batch.reshape(n // 4, 4)
    out = np.empty((n // 4, 4, ML2), WIRE8)
    carry = np.zeros((n // 4, ML2), np.float32)
    for k in range(4):
        tgt = v[:, k] + carry
        q = tgt.astype(WIRE8)
        out[:, k] = q
        if k < 3:
            same = (b4[:, k] == b4[:, k + 1]).astype(np.float32)[:, None]
            carry = (tgt - q.astype(np.float32)) * same
    return out.reshape(n, ML2)


def kernel(node_embedding, W1, b1, W2, b2, W3, b3, w_l2, batch, natoms):
    node_embedding = np.asarray(node_embedding, dtype=np.float32)
    W1 = np.asarray(W1, dtype=np.float32)
    b1 = np.asarray(b1, dtype=np.float32)
    W2 = np.asarray(W2, dtype=np.float32)
    b2 = np.asarray(b2, dtype=np.float32)
    W3 = np.asarray(W3, dtype=np.float32)
    b3 = np.asarray(b3, dtype=np.float32)
    w_l2 = np.asarray(w_l2, dtype=np.float32)
    batch = np.asarray(batch).astype(np.int64)
    natoms_in = np.asarray(natoms)

    N = node_embedding.shape[0]
    G = natoms_in.shape[0]
    n_sh = (N + N_CORES - 1) // N_CORES
    n_sh = ((n_sh + 3) // 4) * 4       # chain alignment
    n_groups = (n_sh + NG - 1) // NG
    n_pad = n_groups * NG
    n_sc = (n_groups + SCG - 1) // SCG

    if N % 4 != 0:
        return _host_reference(node_embedding, W1, b1, W2, b2, W3, b3,
                               w_l2, batch, natoms_in)

    # per-core shard ranges and per-group base graph ids
    shards = []
    W_need = 8
    for c in range(N_CORES):
        n0 = min(c * n_sh, N)
        n1 = min(n0 + n_sh, N)
        b = batch[n0:n1]
        nreal = n1 - n0
        gbase = np.zeros(n_groups, np.int64)
        for grp in range(n_groups):
            lo = grp * NG
            hi = min(lo + NG, nreal)
            if lo < nreal:
                gbase[grp] = b[lo]
                span = int(b[hi - 1] - b[lo] + 1)
                W_need = max(W_need, span)
        shards.append((n0, n1, b, gbase))
    W = _next_pow2(W_need)
    if (W > 32 or not np.all(batch[:-1] <= batch[1:])
            or batch.min(initial=0) < 0 or batch.max(initial=0) >= G):
        return _host_reference(node_embedding, W1, b1, W2, b2, W3, b3,
                               w_l2, batch, natoms_in)

    nc = _build(n_groups, W, n_sh)

    WIRE16 = ml_dtypes.bfloat16
    w1t = np.ascontiguousarray(W1.T).astype(WIRE16)
    w2t = np.ascontiguousarray(W2.T).astype(WIRE16)
    w3t = np.ascontiguousarray(np.repeat(W3.T, 2, axis=1)).astype(WIRE16)
    b1c = np.ascontiguousarray(b1[:, None])
    b2c = np.ascontiguousarray(b2[:, None])
    iota_c = np.ascontiguousarray(
        np.tile(np.arange(W, dtype=np.float32), (P, 1)))

    # global chain-compensated fp8 of the l=2 block + plain fp8 of x0
    el2q = _chain4_quant(
        node_embedding[:, 4:9, :].reshape(N, ML2), batch)
    x0q = node_embedding[:, 0, :].astype(WIRE8)

    in_maps = []
    for c in range(N_CORES):
        n0, n1, b, gbase = shards[c]
        nreal = n1 - n0
        n_pad_sc = n_sc * SCG * NG
        # x0 wire: [sc, c(128), node] channel-major
        x0T = np.zeros((P, n_pad_sc), WIRE8)
        x0T[:, :nreal] = x0q[n0:n1].T
        x0w = np.ascontiguousarray(
            x0T.reshape(P, n_sc, SCG * NG).transpose(1, 0, 2))
        # el2 wire: node j = g*1024 + d*256 + 2p + i lives at
        # [sc, p, (gl, d, h, i, f320)]
        el2 = np.zeros((n_pad_sc, ML2), WIRE8)
        el2[:nreal] = el2q[n0:n1]
        el2 = el2.reshape(n_sc, SCG, DT, P, 2, 2, 320)
        el2 = np.ascontiguousarray(el2.transpose(0, 3, 1, 2, 5, 4, 6)
                                   .reshape(n_sc, P, SCG * GPB))
        # local graph ids per (g, d, p, i)
        lg = np.full(n_pad, -1.0, np.float32)
        lg[:nreal] = (b - np.repeat(gbase, NG)[:nreal]).astype(np.float32)
        lg_t = np.ascontiguousarray(
            lg.reshape(n_groups, DT, P, 2).transpose(2, 0, 1, 3)
              .reshape(P, n_groups * DT * 2))
        in_maps.append({
            "el2w": el2, "x0w": x0w, "lgid": lg_t, "iota_in": iota_c,
            "w1t": w1t, "w2t": w2t, "w3t": w3t, "b1c": b1c, "b2c": b2c,
        })

    res = bass_utils.run_bass_kernel_spmd(nc, in_maps,
                                          core_ids=list(range(N_CORES)))

    # ---- host epilogue ----
    inv = (1.0 / natoms_in.astype(np.float32)).astype(np.float32)
    node_scalar = np.empty(N, np.float32)
    Sfull = np.zeros((G + 32, ML2), np.float32)
    for c in range(N_CORES):
        n0, n1, _, gbase = shards[c]
        nreal = n1 - n0
        Sc = np.asarray(res.results[c]["S_out"]).astype(np.float32)
        # (n_sc, 34, SCG*1024): row 32 = per-node scalars, rows 0:W = S
        node_scalar[n0:n1] = Sc[:, 32, :].reshape(-1)[:nreal]
        for grp in range(n_groups):
            if grp * NG < nreal:
                gb = int(gbase[grp])
                blk = Sc[grp // SCG][:, (grp % SCG) * NG:
                                     (grp % SCG + 1) * NG]
                Sfull[gb:gb + W, 0:320] += blk[0:W, 0:320]
                Sfull[gb:gb + W, 320:640] += blk[0:W, 512:832]
    iso = np.bincount(batch, weights=node_scalar + b3[0], minlength=G)
    iso = iso.astype(np.float32) * inv
    aniso = (Sfull[:G].reshape(G, 5, P) @ w_l2[0]).astype(np.float32)
    aniso *= inv[:, None]
    dec = np.concatenate([iso[:, None], np.zeros((G, 3), np.float32), aniso],
                         axis=1)
    return (dec @ _CG).reshape(-1, 3, 3).astype(np.float32)


# revision 28
# speedup vs baseline: 1.4496x; 1.1276x over previous
"""CartBasisStressHead kernel for Trainium2 (8 NeuronCores, SPMD data-parallel).

Strategy
--------
Only 6 of the 9 m-rows of node_embedding are used: row 0 feeds a SiLU MLP
(per-node scalar), rows 4:9 feed a per-channel contraction (l=2 branch).
Nodes are sharded contiguously across 8 cores; segment sums are linear, so
the host adds per-shard partials.

The kernel is HBM-bound, so everything streamed is wired in fp8 (E4M3):
  * l=2 data uses a chain-of-4 compensated quantization (each node's rounding
    residual is folded into the next node of the same graph before rounding),
    cutting the segment-sum quantization error ~2x below plain fp8 rounding.
  * The segment sum itself runs on the PE in DoubleRow fp8 perf mode:
    256 nodes per pass (128 partitions x 2 pair lanes), with a 0/1 indicator
    matrix A[node, local_graph] as the stationary operand. DoubleRow only
    supports PSUM quadrant 0, so the two 320-column halves of the l=2
    features accumulate into two separate PSUM tiles.
  * The MLP runs x0 (fp8) against bf16 weights; per-node scalars come from a
    1-wide W3 matmul packed into spare PE column bands.

Inputs stream as 5-group superchunks, each split into a 1-group head DMA
(fast pipeline ramp) plus a 4-group rest DMA (~2.6 MB, near-peak HBM
efficiency); outputs stage in SBUF and store once per superchunk. Per-group
PE issue order interleaves the l=2 passes around the MLP layers so the
in-order PE queue never waits on the activation engine.

Host epilogue: scatter-add of per-group segment partials, contraction with
w_l2, bincount of per-node scalars, and the tiny (G,9)@(9,9) basis change.
"""

import sys

if "/opt/trn_rl_repo" not in sys.path:
    sys.path.insert(0, "/opt/trn_rl_repo")

import numpy as np
import ml_dtypes

import concourse.bacc as bacc
import concourse.tile as tile
from concourse import mybir
from concourse import bass_utils

_S2 = 2.0 ** -0.5
_S3 = 3.0 ** -0.5
_S6 = 6.0 ** -0.5
_CG = np.array([
    [_S3, 0, 0, 0, _S3, 0, 0, 0, _S3],
    [0, 0, 0, 0, 0, _S2, 0, -_S2, 0],
    [0, 0, -_S2, 0, 0, 0, _S2, 0, 0],
    [0, _S2, 0, -_S2, 0, 0, 0, 0, 0],
    [0, 0, _S2, 0, 0, 0, _S2, 0, 0],
    [0, 0, 0, 0, 0, _S2, 0, _S2, 0],
    [-_S6, 0, 0, 0, 2 * _S6, 0, 0, 0, -_S6],
    [0, _S2, 0, _S2, 0, 0, 0, 0, 0],
    [-_S2, 0, 0, 0, 0, 0, 0, 0, _S2],
], dtype=np.float32)  # (9, 9)

N_CORES = 8
P = 128          # SBUF partitions
NG = 1024        # real nodes per group (one PSUM accumulation span)
DT = 2           # 256-supernode dtiles per group (supernode = 2 real nodes)
ML2 = 640        # l=2 values per node (5 m-rows x 128 channels)
SCG = 5          # groups per superchunk (one input DMA / output store)
GPB = 2560       # el2 bytes per partition per group (512 supernodes)

F32 = mybir.dt.float32
BF16 = mybir.dt.bfloat16
F8 = mybir.dt.float8e4
DR = mybir.MatmulPerfMode.DoubleRow
WIRE8 = ml_dtypes.float8_e4m3

_BUILD_CACHE = {}


def _build(n_groups, W, n_real):
    key = (n_groups, W, n_real)
    if key in _BUILD_CACHE:
        return _BUILD_CACHE[key]

    n_sc = (n_groups + SCG - 1) // SCG
    T2 = n_groups * DT * 2

    nc = bacc.Bacc("TRN2", target_bir_lowering=False, debug=False,
                   num_devices=N_CORES)

    # inputs (host pre-packed; see kernel() for layouts)
    el2w = nc.dram_tensor("el2w", (n_sc, P, SCG * GPB), F8,
                          kind="ExternalInput").ap()
    x0w = nc.dram_tensor("x0w", (n_sc, P, SCG * NG), F8,
                         kind="ExternalInput").ap()
    lgid = nc.dram_tensor("lgid", (P, T2), F32, kind="ExternalInput").ap()
    iota_in = nc.dram_tensor("iota_in", (P, W), F32, kind="ExternalInput").ap()
    w1t = nc.dram_tensor("w1t", (P, P), BF16, kind="ExternalInput").ap()
    w2t = nc.dram_tensor("w2t", (P, P), BF16, kind="ExternalInput").ap()
    w3t = nc.dram_tensor("w3t", (P, 2), BF16, kind="ExternalInput").ap()
    b1 = nc.dram_tensor("b1c", (P, 1), F32, kind="ExternalInput").ap()
    b2 = nc.dram_tensor("b2c", (P, 1), F32, kind="ExternalInput").ap()
    # output: rows 0:W = l=2 segment partials (halves at cols i*512+0:320),
    # row 32 = per-node scalars (chunk s at cols s*512:...)
    S_out = nc.dram_tensor("S_out", (n_sc, 34, SCG * NG), BF16,
                           kind="ExternalOutput").ap()

    silu = mybir.ActivationFunctionType.Silu
    eq = mybir.AluOpType.is_equal

    with tile.TileContext(nc) as tc:
        with (
            tc.tile_pool(name="const", bufs=1) as cpool,
            tc.tile_pool(name="el2h", bufs=3) as el2hp,
            tc.tile_pool(name="el2r", bufs=3) as el2rp,
            tc.tile_pool(name="x0p", bufs=3) as x0p,
            tc.tile_pool(name="hp", bufs=4) as hp,
            tc.tile_pool(name="stp", bufs=2) as stp,
            tc.tile_pool(name="ph1", bufs=1, space="PSUM") as ph1p,
            tc.tile_pool(name="ph2", bufs=1, space="PSUM") as ph2p,
            tc.tile_pool(name="pS", bufs=2, space="PSUM") as pSp,
        ):
            w1s = cpool.tile([P, P], BF16)
            w2s = cpool.tile([P, P], BF16)
            w3s = cpool.tile([P, 2], BF16)
            b1s = cpool.tile([P, 1], F32)
            b2s = cpool.tile([P, 1], F32)
            iotas = cpool.tile([P, W], F32)
            lgids = cpool.tile([P, T2], F32)
            # every input rides the sync queue (the scalar-ring HWDGE is
            # strictly deprioritized behind sync); consts first, then all
            # superchunks [x0, el2 head, el2 rest] — the sync engine's
            # in-order buffer waits provide natural flow control
            nc.sync.dma_start(out=w1s[:], in_=w1t)
            nc.sync.dma_start(out=w2s[:], in_=w2t)
            nc.sync.dma_start(out=w3s[:], in_=w3t)
            nc.sync.dma_start(out=b1s[:], in_=b1)
            nc.sync.dma_start(out=b2s[:], in_=b2)
            nc.sync.dma_start(out=lgids[:], in_=lgid)
            nc.sync.dma_start(out=iotas[:], in_=iota_in)

            # all indicator pair-matrices up front (one DVE instruction):
            # Aall[p, (g,d,i), w] = (iota[w] == lgid[p, (g,d,i)])
            Aall = cpool.tile([P, T2 * W], F8)
            nc.vector.tensor_tensor(
                out=Aall[:].rearrange("p (t w) -> p t w", t=T2, w=W),
                in0=iotas[:].unsqueeze(1).to_broadcast([P, T2, W]),
                in1=lgids[:].unsqueeze(2).to_broadcast([P, T2, W]),
                op=eq)

            x0cs, el2hs, el2rs = [], [], []
            for sc in range(n_sc):
                sc_g0 = sc * SCG
                sc_ng = min(SCG, n_groups - sc_g0)
                last_g = sc_g0 + sc_ng - 1
                lg_real = min(NG, n_real - last_g * NG)
                dt_last = (lg_real + 511) // 512   # 256-supernode dtiles
                sr_last = (lg_real + 511) // 512
                x0c = x0p.tile([P, SCG * NG], F8, tag="x0c")
                xext = (sc_ng - 1) * NG + sr_last * 512
                nc.sync.dma_start(out=x0c[:, :xext], in_=x0w[sc][:, :xext])
                el2h = el2hp.tile([P, GPB], F8, tag="el2h")
                hext = GPB if sc_ng > 1 else dt_last * 1280
                nc.sync.dma_start(out=el2h[:, :hext],
                                  in_=el2w[sc][:, :hext])
                el2r = el2rp.tile([P, (SCG - 1) * GPB], F8, tag="el2r")
                if sc_ng > 1:
                    ext = (sc_ng - 2) * GPB + dt_last * 1280
                    nc.sync.dma_start(out=el2r[:, :ext],
                                      in_=el2w[sc][:, GPB: GPB + ext])
                x0cs.append(x0c)
                el2hs.append(el2h)
                el2rs.append(el2r)

            # software-pipelined compute: iteration `it` issues
            #   L1(it) -> L2(it-1) -> l2(it) -> W3(it-1) -> copy(it-1)
            # so every matmul's cross-engine inputs are a full group old and
            # the in-order PE queue never stalls on the activation engine.
            # Matmuls sharing a stationary are paired; the second reuses the
            # PE-resident weights (ldweights=False).
            Ssts = {}
            h1map, h2map, pSmap = {}, {}, {}
            for it in range(n_groups + 1):
                if it < n_groups:
                    g = it
                    sc = g // SCG
                    gl = g % SCG
                    grp_real = min(NG, n_real - g * NG)
                    Sr = (grp_real + 511) // 512
                    if gl == 0:
                        Sst = stp.tile([34, SCG * NG], BF16, tag="Sst")
                        Ssts[sc] = Sst

                    # ---- MLP layer 1 of group g ----
                    h1p = ph1p.tile([P, 1024], F32, tag="h1p")
                    for s in range(Sr):
                        nsl = slice(gl * NG + s * 512, gl * NG + (s + 1) * 512)
                        mm = nc.tensor.matmul(h1p[:, s * 512:(s + 1) * 512],
                                              w1s[:], x0cs[sc][:, nsl],
                                              start=True, stop=True)
                        if s > 0:
                            mm.ldweights = False
                    h1s = hp.tile([P, 1024], BF16, tag="h1s")
                    nc.scalar.activation(h1s[:, :Sr * 512],
                                         h1p[:, :Sr * 512], silu, bias=b1s[:])
                    h1map[g] = (h1s, Sr)

                if it >= 1:
                    # ---- MLP layer 2 of group it-1 ----
                    gp = it - 1
                    h1s_p, Sr_p = h1map.pop(gp)
                    h2p = ph2p.tile([P, 1024], F32, tag="h2p")
                    for s in range(Sr_p):
                        mm = nc.tensor.matmul(
                            h2p[:, s * 512:(s + 1) * 512], w2s[:],
                            h1s_p[:, s * 512:(s + 1) * 512],
                            start=True, stop=True)
                        if s > 0:
                            mm.ldweights = False
                    h2s = hp.tile([P, 1024], BF16, tag="h2s")
                    nc.scalar.activation(h2s[:, :Sr_p * 512],
                                         h2p[:, :Sr_p * 512], silu,
                                         bias=b2s[:])
                    h2map[gp] = (h2s, Sr_p)

                if it < n_groups:
                    # ---- l=2 segment sum of group g (DoubleRow fp8) ----
                    # operates on supernodes (host pre-summed node pairs);
                    # one 2-bank PSUM tile per group:
                    #   [0:W, 0:320]    l=2 feature half 0 (bank A)
                    #   [0:W, 512:832]  l=2 feature half 1 (bank B)
                    #   [32:34, 0:512]  W3 scalars chunk 0 (dup rows)
                    #   [32:34, 512:1024] W3 scalars chunk 1
                    Dr = (grp_real + 511) // 512
                    pS = pSp.tile([P, 1024], F32, tag="pS")
                    if gl == 0:
                        esrc = el2hs[sc]
                        ebase = 0
                    else:
                        esrc = el2rs[sc]
                        ebase = (gl - 1) * GPB
                    for d in range(Dr):
                        t2i = (g * DT + d) * 2
                        Ad = Aall[:, t2i * W: (t2i + 2) * W] \
                            .rearrange("p (i w) -> p i w", i=2, w=W)
                        base = ebase + d * 1280
                        nc.tensor.matmul(
                            pS[0:W, 0:320], Ad,
                            esrc[:, base: base + 640]
                                .rearrange("p (i f) -> p i f", i=2, f=320),
                            start=(d == 0), stop=(d == Dr - 1),
                            perf_mode=DR, tile_position=(0, 0))
                        mm = nc.tensor.matmul(
                            pS[0:W, 512:832], Ad,
                            esrc[:, base + 640: base + 1280]
                                .rearrange("p (i f) -> p i f", i=2, f=320),
                            start=(d == 0), stop=(d == Dr - 1),
                            perf_mode=DR, tile_position=(0, 0))
                        mm.ldweights = False
                    pSmap[g] = pS

                if it >= 1:
                    # ---- per-node scalars + staging copy of group it-1 ----
                    gp = it - 1
                    h2s_p, Sr_p = h2map.pop(gp)
                    pS_p = pSmap.pop(gp)
                    for s in range(Sr_p):
                        mm = nc.tensor.matmul(
                            pS_p[32:34, s * 512:(s + 1) * 512], w3s[:],
                            h2s_p[:, s * 512:(s + 1) * 512],
                            start=True, stop=True, tile_position=(0, 32))
                        if s > 0:
                            mm.ldweights = False
                    scp = gp // SCG
                    glp = gp % SCG
                    nc.vector.tensor_copy(
                        out=Ssts[scp][:, glp * NG: (glp + 1) * NG],
                        in_=pS_p[0:34, 0:1024])
                    if gp == n_groups - 1 or glp == SCG - 1:
                        nc.gpsimd.dma_start(out=S_out[scp],
                                            in_=Ssts.pop(scp)[:])

    nc.compile()
    _BUILD_CACHE[key] = nc
    return nc


def _next_pow2(x):
    p = 8
    while p < x:
        p *= 2
    return p


def _host_reference(node_embedding, W1, b1, W2, b2, W3, b3, w_l2, batch,
                    natoms):
    """Pure-numpy fallback (only used for pathological graph layouts)."""
    G = natoms.shape[0]
    inv = 1.0 / natoms.astype(np.float32)
    x = node_embedding[:, 0, :]
    h = x @ W1.T + b1
    h = h / (1.0 + np.exp(-h))
    h = h @ W2.T + b2
    h = h / (1.0 + np.exp(-h))
    ns = (h @ W3.T + b3)[:, 0]
    ok = (batch >= 0) & (batch < G)
    bok = batch[ok]
    iso = np.bincount(bok, weights=ns[ok], minlength=G).astype(np.float32) \
        * inv
    nl2 = np.einsum("nmc,c->nm", node_embedding[:, 4:9, :], w_l2[0])
    aniso = np.stack(
        [np.bincount(bok, weights=nl2[ok, m], minlength=G)
         for m in range(5)], axis=1).astype(np.float32) * inv[:, None]
    dec = np.concatenate([iso[:, None], np.zeros((G, 3), np.float32), aniso],
                         axis=1)
    return (dec @ _CG).reshape(-1, 3, 3).astype(np.float32)


def _chain4_quant(vals, seg):
    """fp8 E4M3 quantization with 4-element error-feedback chains.

    Within each aligned run of 4 rows, the rounding residual of row k is
    added to row k+1 before its rounding whenever both rows belong to the
    same segment, so the segment sum sees ~1 rounding error per chain
    instead of 4."""
    n = vals.shape[0]
    v = vals.reshape(n // 4, 4, ML2)
    b4 = seg.reshape(n // 4, 4)
    out = np.empty((n // 4, 4, ML2), WIRE8)
    carry = np.zeros((n // 4, ML2), np.float32)
    for k in range(4):
        tgt = v[:, k] + carry
        q = tgt.astype(WIRE8)
        out[:, k] = q
        if k < 3:
            same = (b4[:, k] == b4[:, k + 1]).astype(np.float32)[:, None]
            carry = (tgt - q.astype(np.float32)) * same
    return out.reshape(n, ML2)


def kernel(node_embedding, W1, b1, W2, b2, W3, b3, w_l2, batch, natoms):
    node_embedding = np.asarray(node_embedding, dtype=np.float32)
    W1 = np.asarray(W1, dtype=np.float32)
    b1 = np.asarray(b1, dtype=np.float32)
    W2 = np.asarray(W2, dtype=np.float32)
    b2 = np.asarray(b2, dtype=np.float32)
    W3 = np.asarray(W3, dtype=np.float32)
    b3 = np.asarray(b3, dtype=np.float32)
    w_l2 = np.asarray(w_l2, dtype=np.float32)
    batch = np.asarray(batch).astype(np.int64)
    natoms_in = np.asarray(natoms)

    N = node_embedding.shape[0]
    G = natoms_in.shape[0]
    n_sh = (N + N_CORES - 1) // N_CORES
    n_sh = ((n_sh + 3) // 4) * 4       # chain alignment
    n_groups = (n_sh + NG - 1) // NG
    n_pad = n_groups * NG
    n_sc = (n_groups + SCG - 1) // SCG

    if N % 8 != 0:
        return _host_reference(node_embedding, W1, b1, W2, b2, W3, b3,
                               w_l2, batch, natoms_in)

    # per-core shard ranges and per-group base graph ids
    shards = []
    W_need = 8
    for c in range(N_CORES):
        n0 = min(c * n_sh, N)
        n1 = min(n0 + n_sh, N)
        b = batch[n0:n1]
        nreal = n1 - n0
        gbase = np.zeros(n_groups, np.int64)
        for grp in range(n_groups):
            lo = grp * NG
            hi = min(lo + NG, nreal)
            if lo < nreal:
                gbase[grp] = b[lo]
                span = int(b[hi - 1] - b[lo] + 1)
                W_need = max(W_need, span)
        shards.append((n0, n1, b, gbase))
    W = _next_pow2(W_need)
    if (W > 32 or not np.all(batch[:-1] <= batch[1:])
            or batch.min(initial=0) < 0 or batch.max(initial=0) >= G):
        return _host_reference(node_embedding, W1, b1, W2, b2, W3, b3,
                               w_l2, batch, natoms_in)

    nc = _build(n_groups, W, n_sh)

    WIRE16 = ml_dtypes.bfloat16
    w1t = np.ascontiguousarray(W1.T).astype(WIRE16)
    w2t = np.ascontiguousarray(W2.T).astype(WIRE16)
    w3t = np.ascontiguousarray(np.repeat(W3.T, 2, axis=1)).astype(WIRE16)
    b1c = np.ascontiguousarray(b1[:, None])
    b2c = np.ascontiguousarray(b2[:, None])
    iota_c = np.ascontiguousarray(
        np.tile(np.arange(W, dtype=np.float32), (P, 1)))

    # l=2 branch runs on supernodes: adjacent node pairs pre-summed on the
    # host when both land in the same graph; when a pair straddles a graph
    # boundary the odd node is exiled to an exact host-side correction.
    el2n = node_embedding[:, 4:9, :].reshape(N, ML2)
    bs = batch[0::2]
    same_pair = (bs == batch[1::2])
    sn = el2n[0::2] + el2n[1::2] * same_pair[:, None].astype(np.float32)
    el2q = _chain4_quant(sn, bs)
    x0q = node_embedding[:, 0, :].astype(WIRE8)

    in_maps = []
    for c in range(N_CORES):
        n0, n1, b, gbase = shards[c]
        nreal = n1 - n0
        n_pad_sc = n_sc * SCG * NG
        # x0 wire: [sc, c(128), node] channel-major
        x0T = np.zeros((P, n_pad_sc), WIRE8)
        x0T[:, :nreal] = x0q[n0:n1].T
        x0w = np.ascontiguousarray(
            x0T.reshape(P, n_sc, SCG * NG).transpose(1, 0, 2))
        # el2 wire over supernodes: supernode j = g*512 + d*256 + 2p + i
        # lives at [sc, p, (gl, d, h, i, f320)]
        sreal = nreal // 2
        el2 = np.zeros((n_pad_sc // 2, ML2), WIRE8)
        el2[:sreal] = el2q[n0 // 2: n1 // 2]
        el2 = el2.reshape(n_sc, SCG, DT, P, 2, 2, 320)
        el2 = np.ascontiguousarray(el2.transpose(0, 3, 1, 2, 5, 4, 6)
                                   .reshape(n_sc, P, SCG * GPB))
        # local graph ids per supernode (g, d, p, i)
        lg = np.full(n_pad // 2, -1.0, np.float32)
        lg[:sreal] = (bs[n0 // 2: n1 // 2]
                      - np.repeat(gbase, NG // 2)[:sreal]).astype(np.float32)
        lg_t = np.ascontiguousarray(
            lg.reshape(n_groups, DT, P, 2).transpose(2, 0, 1, 3)
              .reshape(P, n_groups * DT * 2))
        in_maps.append({
            "el2w": el2, "x0w": x0w, "lgid": lg_t, "iota_in": iota_c,
            "w1t": w1t, "w2t": w2t, "w3t": w3t, "b1c": b1c, "b2c": b2c,
        })

    res = bass_utils.run_bass_kernel_spmd(nc, in_maps,
                                          core_ids=list(range(N_CORES)))

    # ---- host epilogue ----
    inv = (1.0 / natoms_in.astype(np.float32)).astype(np.float32)
    node_scalar = np.empty(N, np.float32)
    Sfull = np.zeros((G + 32, ML2), np.float32)
    for c in range(N_CORES):
        n0, n1, _, gbase = shards[c]
        nreal = n1 - n0
        Sc = np.asarray(res.results[c]["S_out"]).astype(np.float32)
        # (n_sc, 34, SCG*1024): row 32 = per-node scalars, rows 0:W = S
        node_scalar[n0:n1] = Sc[:, 32, :].reshape(-1)[:nreal]
        for grp in range(n_groups):
            if grp * NG < nreal:
                gb = int(gbase[grp])
                blk = Sc[grp // SCG][:, (grp % SCG) * NG:
                                     (grp % SCG + 1) * NG]
                Sfull[gb:gb + W, 0:320] += blk[0:W, 0:320]
                Sfull[gb:gb + W, 320:640] += blk[0:W, 512:832]
    iso = np.bincount(batch, weights=node_scalar + b3[0], minlength=G)
    iso = iso.astype(np.float32) * inv
    aniso = (Sfull[:G].reshape(G, 5, P) @ w_l2[0]).astype(np.float32)
    # exact host-side correction for nodes exiled from straddling pairs
    ex = np.nonzero(~same_pair)[0]
    if ex.size:
        exn = 2 * ex + 1
        nl2_ex = np.einsum("nmc,c->nm", node_embedding[exn, 4:9, :], w_l2[0])
        bex = batch[exn]
        for m in range(5):
            aniso[:, m] += np.bincount(
                bex, weights=nl2_ex[:, m], minlength=G).astype(np.float32)
    aniso *= inv[:, None]
    dec = np.concatenate([iso[:, None], np.zeros((G, 3), np.float32), aniso],
                         axis=1)
    return (dec @ _CG).reshape(-1, 3, 3).astype(np.float32)


# revision 37
# speedup vs baseline: 1.6662x; 1.1494x over previous
"""CartBasisStressHead kernel for Trainium2 (8 NeuronCores, SPMD data-parallel).

Strategy
--------
Only 6 of the 9 m-rows of node_embedding are used: row 0 feeds a SiLU MLP
(per-node scalar), rows 4:9 feed a per-channel contraction (l=2 branch).
Nodes are sharded contiguously across 8 cores; segment sums are linear, so
the host adds per-shard partials.

The kernel is HBM-bound, so everything streamed is wired in fp8 (E4M3):
  * l=2 data uses a chain-of-4 compensated quantization (each node's rounding
    residual is folded into the next node of the same graph before rounding),
    cutting the segment-sum quantization error ~2x below plain fp8 rounding.
  * The segment sum itself runs on the PE in DoubleRow fp8 perf mode:
    256 nodes per pass (128 partitions x 2 pair lanes), with a 0/1 indicator
    matrix A[node, local_graph] as the stationary operand. DoubleRow only
    supports PSUM quadrant 0, so the two 320-column halves of the l=2
    features accumulate into two separate PSUM tiles.
  * The MLP runs x0 (fp8) against bf16 weights; per-node scalars come from a
    1-wide W3 matmul packed into spare PE column bands.

Inputs stream as 5-group superchunks, each split into a 1-group head DMA
(fast pipeline ramp) plus a 4-group rest DMA (~2.6 MB, near-peak HBM
efficiency); outputs stage in SBUF and store once per superchunk. Per-group
PE issue order interleaves the l=2 passes around the MLP layers so the
in-order PE queue never waits on the activation engine.

Host epilogue: scatter-add of per-group segment partials, contraction with
w_l2, bincount of per-node scalars, and the tiny (G,9)@(9,9) basis change.
"""

import sys

if "/opt/trn_rl_repo" not in sys.path:
    sys.path.insert(0, "/opt/trn_rl_repo")

import numpy as np
import ml_dtypes

import concourse.bacc as bacc
import concourse.tile as tile
from concourse import mybir
from concourse import bass_utils

_S2 = 2.0 ** -0.5
_S3 = 3.0 ** -0.5
_S6 = 6.0 ** -0.5
_CG = np.array([
    [_S3, 0, 0, 0, _S3, 0, 0, 0, _S3],
    [0, 0, 0, 0, 0, _S2, 0, -_S2, 0],
    [0, 0, -_S2, 0, 0, 0, _S2, 0, 0],
    [0, _S2, 0, -_S2, 0, 0, 0, 0, 0],
    [0, 0, _S2, 0, 0, 0, _S2, 0, 0],
    [0, 0, 0, 0, 0, _S2, 0, _S2, 0],
    [-_S6, 0, 0, 0, 2 * _S6, 0, 0, 0, -_S6],
    [0, _S2, 0, _S2, 0, 0, 0, 0, 0],
    [-_S2, 0, 0, 0, 0, 0, 0, 0, _S2],
], dtype=np.float32)  # (9, 9)

N_CORES = 8
P = 128          # SBUF partitions
NG = 1024        # real nodes per group (one PSUM accumulation span)
DT = 2           # 256-supernode dtiles per group (supernode = 2 real nodes)
ML2 = 640        # l=2 values per node (5 m-rows x 128 channels)
SCG = 5          # groups per superchunk (one input DMA / output store)
GPB = 2560       # el2 bytes per partition per group (512 supernodes)

F32 = mybir.dt.float32
BF16 = mybir.dt.bfloat16
F8 = mybir.dt.float8e4
DR = mybir.MatmulPerfMode.DoubleRow
WIRE8 = ml_dtypes.float8_e4m3

_BUILD_CACHE = {}


def _build(n_groups, W, n_real):
    key = (n_groups, W, n_real)
    if key in _BUILD_CACHE:
        return _BUILD_CACHE[key]

    n_sc = (n_groups + SCG - 1) // SCG
    T2 = n_groups * DT * 2

    nc = bacc.Bacc("TRN2", target_bir_lowering=False, debug=False,
                   num_devices=N_CORES)

    # inputs (host pre-packed; see kernel() for layouts)
    el2w = nc.dram_tensor("el2w", (n_sc, P, SCG * GPB), F8,
                          kind="ExternalInput").ap()
    x0w = nc.dram_tensor("x0w", (n_sc, P, SCG * NG), F8,
                         kind="ExternalInput").ap()
    lgid = nc.dram_tensor("lgid", (P, T2), F32, kind="ExternalInput").ap()
    iota_in = nc.dram_tensor("iota_in", (P, W), F32, kind="ExternalInput").ap()
    w1t = nc.dram_tensor("w1t", (P, P), BF16, kind="ExternalInput").ap()
    w2t = nc.dram_tensor("w2t", (P, P), BF16, kind="ExternalInput").ap()
    b1 = nc.dram_tensor("b1c", (P, 1), F32, kind="ExternalInput").ap()
    b2 = nc.dram_tensor("b2c", (P, 1), F32, kind="ExternalInput").ap()
    # outputs: l=2 segment partials (rows 0:W, halves at i*320), and the
    # fp8 hidden activations (host contracts with W3 for the scalar branch)
    S_out = nc.dram_tensor("S_out", (n_sc, 32, SCG * 640), BF16,
                           kind="ExternalOutput").ap()
    h2o = nc.dram_tensor("h2o", (n_groups, P, NG), F8,
                         kind="ExternalOutput").ap()

    silu = mybir.ActivationFunctionType.Silu
    eq = mybir.AluOpType.is_equal

    with tile.TileContext(nc) as tc:
        with (
            tc.tile_pool(name="const", bufs=1) as cpool,
            tc.tile_pool(name="el2h", bufs=3) as el2hp,
            tc.tile_pool(name="el2r", bufs=3) as el2rp,
            tc.tile_pool(name="x0p", bufs=3) as x0p,
            tc.tile_pool(name="hp", bufs=4) as hp,
            tc.tile_pool(name="stp", bufs=2) as stp,
            tc.tile_pool(name="ph1", bufs=1, space="PSUM") as ph1p,
            tc.tile_pool(name="ph2", bufs=1, space="PSUM") as ph2p,
            tc.tile_pool(name="pS", bufs=2, space="PSUM") as pSp,
        ):
            w1s = cpool.tile([P, P], BF16)
            w2s = cpool.tile([P, P], BF16)
            b1s = cpool.tile([P, 1], F32)
            b2s = cpool.tile([P, 1], F32)
            iotas = cpool.tile([P, W], F32)
            lgids = cpool.tile([P, T2], F32)
            # every input rides the sync queue (the scalar-ring HWDGE is
            # strictly deprioritized behind sync); consts first, then all
            # superchunks [x0, el2 head, el2 rest] — the sync engine's
            # in-order buffer waits provide natural flow control
            nc.sync.dma_start(out=w1s[:], in_=w1t)
            nc.sync.dma_start(out=w2s[:], in_=w2t)
            nc.sync.dma_start(out=b1s[:], in_=b1)
            nc.sync.dma_start(out=b2s[:], in_=b2)
            nc.sync.dma_start(out=lgids[:], in_=lgid)
            nc.sync.dma_start(out=iotas[:], in_=iota_in)

            # all indicator pair-matrices up front (one DVE instruction):
            # Aall[p, (g,d,i), w] = (iota[w] == lgid[p, (g,d,i)])
            Aall = cpool.tile([P, T2 * W], F8)
            nc.vector.tensor_tensor(
                out=Aall[:].rearrange("p (t w) -> p t w", t=T2, w=W),
                in0=iotas[:].unsqueeze(1).to_broadcast([P, T2, W]),
                in1=lgids[:].unsqueeze(2).to_broadcast([P, T2, W]),
                op=eq)

            x0cs, el2hs, el2rs = [], [], []
            for sc in range(n_sc):
                sc_g0 = sc * SCG
                sc_ng = min(SCG, n_groups - sc_g0)
                last_g = sc_g0 + sc_ng - 1
                lg_real = min(NG, n_real - last_g * NG)
                dt_last = (lg_real + 511) // 512   # 256-supernode dtiles
                sr_last = (lg_real + 511) // 512
                x0c = x0p.tile([P, SCG * NG], F8, tag="x0c")
                xext = (sc_ng - 1) * NG + sr_last * 512
                nc.sync.dma_start(out=x0c[:, :xext], in_=x0w[sc][:, :xext])
                el2h = el2hp.tile([P, GPB], F8, tag="el2h")
                hext = GPB if sc_ng > 1 else dt_last * 1280
                nc.sync.dma_start(out=el2h[:, :hext],
                                  in_=el2w[sc][:, :hext])
                el2r = el2rp.tile([P, (SCG - 1) * GPB], F8, tag="el2r")
                if sc_ng > 1:
                    ext = (sc_ng - 2) * GPB + dt_last * 1280
                    nc.sync.dma_start(out=el2r[:, :ext],
                                      in_=el2w[sc][:, GPB: GPB + ext])
                x0cs.append(x0c)
                el2hs.append(el2h)
                el2rs.append(el2r)

            # software-pipelined compute: iteration `it` issues
            #   L1(it) -> L2(it-1) -> l2(it) -> W3(it-1) -> copy(it-1)
            # so every matmul's cross-engine inputs are a full group old and
            # the in-order PE queue never stalls on the activation engine.
            # Matmuls sharing a stationary are paired; the second reuses the
            # PE-resident weights (ldweights=False).
            Ssts = {}
            h1map, h2map, pSmap = {}, {}, {}
            for it in range(n_groups + 1):
                if it < n_groups:
                    g = it
                    sc = g // SCG
                    gl = g % SCG
                    grp_real = min(NG, n_real - g * NG)
                    Sr = (grp_real + 511) // 512
                    if gl == 0:
                        Sst = stp.tile([32, SCG * 640], BF16, tag="Sst")
                        Ssts[sc] = Sst

                    # ---- MLP layer 1 of group g ----
                    h1p = ph1p.tile([P, 1024], F32, tag="h1p")
                    for s in range(Sr):
                        nsl = slice(gl * NG + s * 512, gl * NG + (s + 1) * 512)
                        mm = nc.tensor.matmul(h1p[:, s * 512:(s + 1) * 512],
                                              w1s[:], x0cs[sc][:, nsl],
                                              start=True, stop=True)
                        if s > 0:
                            mm.ldweights = False
                    h1s = hp.tile([P, 1024], BF16, tag="h1s")
                    nc.scalar.activation(h1s[:, :Sr * 512],
                                         h1p[:, :Sr * 512], silu, bias=b1s[:])
                    h1map[g] = (h1s, Sr)

                if it >= 1:
                    # ---- MLP layer 2 of group it-1; fp8 h2 to the host ----
                    gp = it - 1
                    h1s_p, Sr_p = h1map.pop(gp)
                    h2p = ph2p.tile([P, 1024], F32, tag="h2p")
                    for s in range(Sr_p):
                        nc.tensor.matmul(
                            h2p[:, s * 512:(s + 1) * 512], w2s[:],
                            h1s_p[:, s * 512:(s + 1) * 512],
                            start=True, stop=True)
                    h2s = hp.tile([P, 1024], F8, tag="h2s")
                    nc.scalar.activation(h2s[:, :Sr_p * 512],
                                         h2p[:, :Sr_p * 512], silu,
                                         bias=b2s[:])
                    nc.gpsimd.dma_start(out=h2o[gp][:, :Sr_p * 512],
                                        in_=h2s[:, :Sr_p * 512])

                if it < n_groups:
                    # ---- l=2 segment sum of group g (DoubleRow fp8) ----
                    # operates on supernodes (host pre-summed node pairs);
                    # one 2-bank PSUM tile per group:
                    #   [0:W, 0:320]    l=2 feature half 0 (bank A)
                    #   [0:W, 512:832]  l=2 feature half 1 (bank B)
                    #   [32:34, 0:512]  W3 scalars chunk 0 (dup rows)
                    #   [32:34, 512:1024] W3 scalars chunk 1
                    Dr = (grp_real + 511) // 512
                    pS = pSp.tile([P, 1024], F32, tag="pS")
                    if gl == 0:
                        esrc = el2hs[sc]
                        ebase = 0
                    else:
                        esrc = el2rs[sc]
                        ebase = (gl - 1) * GPB
                    for d in range(Dr):
                        t2i = (g * DT + d) * 2
                        Ad = Aall[:, t2i * W: (t2i + 2) * W] \
                            .rearrange("p (i w) -> p i w", i=2, w=W)
                        base = ebase + d * 1280
                        nc.tensor.matmul(
                            pS[0:W, 0:320], Ad,
                            esrc[:, base: base + 640]
                                .rearrange("p (i f) -> p i f", i=2, f=320),
                            start=(d == 0), stop=(d == Dr - 1),
                            perf_mode=DR, tile_position=(0, 0))
                        mm = nc.tensor.matmul(
                            pS[0:W, 512:832], Ad,
                            esrc[:, base + 640: base + 1280]
                                .rearrange("p (i f) -> p i f", i=2, f=320),
                            start=(d == 0), stop=(d == Dr - 1),
                            perf_mode=DR, tile_position=(0, 0))
                        mm.ldweights = False
                    pSmap[g] = pS

                if it >= 1:
                    # ---- staging copy of group it-1 (strided, skips pad) ----
                    gp = it - 1
                    pS_p = pSmap.pop(gp)
                    scp = gp // SCG
                    glp = gp % SCG
                    nc.vector.tensor_copy(
                        out=Ssts[scp][:, glp * 640: (glp + 1) * 640]
                            .rearrange("p (i f) -> p i f", i=2, f=320),
                        in_=pS_p[0:32, 0:1024]
                            .rearrange("p (i f) -> p i f", i=2, f=512)
                            [:, :, 0:320])
                    if gp == n_groups - 1 or glp == SCG - 1:
                        nc.gpsimd.dma_start(out=S_out[scp],
                                            in_=Ssts.pop(scp)[:])

    nc.compile()
    _BUILD_CACHE[key] = nc
    return nc


def _next_pow2(x):
    p = 8
    while p < x:
        p *= 2
    return p


def _host_reference(node_embedding, W1, b1, W2, b2, W3, b3, w_l2, batch,
                    natoms):
    """Pure-numpy fallback (only used for pathological graph layouts)."""
    G = natoms.shape[0]
    inv = 1.0 / natoms.astype(np.float32)
    x = node_embedding[:, 0, :]
    h = x @ W1.T + b1
    h = h / (1.0 + np.exp(-h))
    h = h @ W2.T + b2
    h = h / (1.0 + np.exp(-h))
    ns = (h @ W3.T + b3)[:, 0]
    ok = (batch >= 0) & (batch < G)
    bok = batch[ok]
    iso = np.bincount(bok, weights=ns[ok], minlength=G).astype(np.float32) \
        * inv
    nl2 = np.einsum("nmc,c->nm", node_embedding[:, 4:9, :], w_l2[0])
    aniso = np.stack(
        [np.bincount(bok, weights=nl2[ok, m], minlength=G)
         for m in range(5)], axis=1).astype(np.float32) * inv[:, None]
    dec = np.concatenate([iso[:, None], np.zeros((G, 3), np.float32), aniso],
                         axis=1)
    return (dec @ _CG).reshape(-1, 3, 3).astype(np.float32)


def _chain4_quant(vals, seg):
    """fp8 E4M3 quantization with 4-element error-feedback chains.

    Within each aligned run of 4 rows, the rounding residual of row k is
    added to row k+1 before its rounding whenever both rows belong to the
    same segment, so the segment sum sees ~1 rounding error per chain
    instead of 4."""
    n = vals.shape[0]
    v = vals.reshape(n // 4, 4, ML2)
    b4 = seg.reshape(n // 4, 4)
    out = np.empty((n // 4, 4, ML2), WIRE8)
    carry = np.zeros((n // 4, ML2), np.float32)
    for k in range(4):
        tgt = v[:, k] + carry
        q = tgt.astype(WIRE8)
        out[:, k] = q
        if k < 3:
            same = (b4[:, k] == b4[:, k + 1]).astype(np.float32)[:, None]
            carry = (tgt - q.astype(np.float32)) * same
    return out.reshape(n, ML2)


def kernel(node_embedding, W1, b1, W2, b2, W3, b3, w_l2, batch, natoms):
    node_embedding = np.asarray(node_embedding, dtype=np.float32)
    W1 = np.asarray(W1, dtype=np.float32)
    b1 = np.asarray(b1, dtype=np.float32)
    W2 = np.asarray(W2, dtype=np.float32)
    b2 = np.asarray(b2, dtype=np.float32)
    W3 = np.asarray(W3, dtype=np.float32)
    b3 = np.asarray(b3, dtype=np.float32)
    w_l2 = np.asarray(w_l2, dtype=np.float32)
    batch = np.asarray(batch).astype(np.int64)
    natoms_in = np.asarray(natoms)

    N = node_embedding.shape[0]
    G = natoms_in.shape[0]
    n_sh = (N + N_CORES - 1) // N_CORES
    n_sh = ((n_sh + 3) // 4) * 4       # chain alignment
    n_groups = (n_sh + NG - 1) // NG
    n_pad = n_groups * NG
    n_sc = (n_groups + SCG - 1) // SCG

    if N % 8 != 0:
        return _host_reference(node_embedding, W1, b1, W2, b2, W3, b3,
                               w_l2, batch, natoms_in)

    # per-core shard ranges and per-group base graph ids
    shards = []
    W_need = 8
    for c in range(N_CORES):
        n0 = min(c * n_sh, N)
        n1 = min(n0 + n_sh, N)
        b = batch[n0:n1]
        nreal = n1 - n0
        gbase = np.zeros(n_groups, np.int64)
        for grp in range(n_groups):
            lo = grp * NG
            hi = min(lo + NG, nreal)
            if lo < nreal:
                gbase[grp] = b[lo]
                span = int(b[hi - 1] - b[lo] + 1)
                W_need = max(W_need, span)
        shards.append((n0, n1, b, gbase))
    W = _next_pow2(W_need)
    if (W > 32 or not np.all(batch[:-1] <= batch[1:])
            or batch.min(initial=0) < 0 or batch.max(initial=0) >= G):
        return _host_reference(node_embedding, W1, b1, W2, b2, W3, b3,
                               w_l2, batch, natoms_in)

    nc = _build(n_groups, W, n_sh)

    WIRE16 = ml_dtypes.bfloat16
    w1t = np.ascontiguousarray(W1.T).astype(WIRE16)
    w2t = np.ascontiguousarray(W2.T).astype(WIRE16)
    b1c = np.ascontiguousarray(b1[:, None])
    b2c = np.ascontiguousarray(b2[:, None])
    iota_c = np.ascontiguousarray(
        np.tile(np.arange(W, dtype=np.float32), (P, 1)))

    # l=2 branch runs on supernodes: adjacent node pairs pre-summed on the
    # host when both land in the same graph; when a pair straddles a graph
    # boundary the odd node is exiled to an exact host-side correction.
    el2n = node_embedding[:, 4:9, :].reshape(N, ML2)
    bs = batch[0::2]
    same_pair = (bs == batch[1::2])
    sn = el2n[0::2] + el2n[1::2] * same_pair[:, None].astype(np.float32)
    el2q = _chain4_quant(sn, bs)
    x0q = node_embedding[:, 0, :].astype(WIRE8)

    in_maps = []
    for c in range(N_CORES):
        n0, n1, b, gbase = shards[c]
        nreal = n1 - n0
        n_pad_sc = n_sc * SCG * NG
        # x0 wire: [sc, c(128), node] channel-major
        x0T = np.zeros((P, n_pad_sc), WIRE8)
        x0T[:, :nreal] = x0q[n0:n1].T
        x0w = np.ascontiguousarray(
            x0T.reshape(P, n_sc, SCG * NG).transpose(1, 0, 2))
        # el2 wire over supernodes: supernode j = g*512 + d*256 + 2p + i
        # lives at [sc, p, (gl, d, h, i, f320)]
        sreal = nreal // 2
        el2 = np.zeros((n_pad_sc // 2, ML2), WIRE8)
        el2[:sreal] = el2q[n0 // 2: n1 // 2]
        el2 = el2.reshape(n_sc, SCG, DT, P, 2, 2, 320)
        el2 = np.ascontiguousarray(el2.transpose(0, 3, 1, 2, 5, 4, 6)
                                   .reshape(n_sc, P, SCG * GPB))
        # local graph ids per supernode (g, d, p, i)
        lg = np.full(n_pad // 2, -1.0, np.float32)
        lg[:sreal] = (bs[n0 // 2: n1 // 2]
                      - np.repeat(gbase, NG // 2)[:sreal]).astype(np.float32)
        lg_t = np.ascontiguousarray(
            lg.reshape(n_groups, DT, P, 2).transpose(2, 0, 1, 3)
              .reshape(P, n_groups * DT * 2))
        in_maps.append({
            "el2w": el2, "x0w": x0w, "lgid": lg_t, "iota_in": iota_c,
            "w1t": w1t, "w2t": w2t, "b1c": b1c, "b2c": b2c,
        })

    res = bass_utils.run_bass_kernel_spmd(nc, in_maps,
                                          core_ids=list(range(N_CORES)))

    # ---- host epilogue ----
    inv = (1.0 / natoms_in.astype(np.float32)).astype(np.float32)
    node_scalar = np.empty(N, np.float32)
    Sfull = np.zeros((G + 32, ML2), np.float32)
    for c in range(N_CORES):
        n0, n1, _, gbase = shards[c]
        nreal = n1 - n0
        Sc = np.asarray(res.results[c]["S_out"]).astype(np.float32)
        # per-node scalars: host contraction of the fp8 hidden activations
        h2f = np.asarray(res.results[c]["h2o"]).astype(np.float32)
        sc_nodes = np.tensordot(W3[0], h2f, axes=([0], [1])).reshape(-1)
        node_scalar[n0:n1] = sc_nodes[:nreal]
        for grp in range(n_groups):
            if grp * NG < nreal:
                gb = int(gbase[grp])
                blk = Sc[grp // SCG][:, (grp % SCG) * 640:
                                     (grp % SCG + 1) * 640]
                Sfull[gb:gb + W, 0:320] += blk[0:W, 0:320]
                Sfull[gb:gb + W, 320:640] += blk[0:W, 320:640]
    iso = np.bincount(batch, weights=node_scalar + b3[0], minlength=G)
    iso = iso.astype(np.float32) * inv
    aniso = (Sfull[:G].reshape(G, 5, P) @ w_l2[0]).astype(np.float32)
    # exact host-side correction for nodes exiled from straddling pairs
    ex = np.nonzero(~same_pair)[0]
    if ex.size:
        exn = 2 * ex + 1
        nl2_ex = np.einsum("nmc,c->nm", node_embedding[exn, 4:9, :], w_l2[0])
        bex = batch[exn]
        for m in range(5):
            aniso[:, m] += np.bincount(
                bex, weights=nl2_ex[:, m], minlength=G).astype(np.float32)
    aniso *= inv[:, None]
    dec = np.concatenate([iso[:, None], np.zeros((G, 3), np.float32), aniso],
                         axis=1)
    return (dec @ _CG).reshape(-1, 3, 3).astype(np.float32)


# revision 42
# speedup vs baseline: 1.6853x; 1.0115x over previous
"""CartBasisStressHead kernel for Trainium2 (8 NeuronCores, SPMD data-parallel).

Strategy
--------
Only 6 of the 9 m-rows of node_embedding are used: row 0 feeds a SiLU MLP
(per-node scalar), rows 4:9 feed a per-channel contraction (l=2 branch).
Nodes are sharded contiguously across 8 cores; segment sums are linear, so
the host adds per-shard partials.

The kernel is HBM-bound, so everything streamed is wired in fp8 (E4M3):
  * l=2 data uses a chain-of-4 compensated quantization (each node's rounding
    residual is folded into the next node of the same graph before rounding),
    cutting the segment-sum quantization error ~2x below plain fp8 rounding.
  * The segment sum itself runs on the PE in DoubleRow fp8 perf mode:
    256 nodes per pass (128 partitions x 2 pair lanes), with a 0/1 indicator
    matrix A[node, local_graph] as the stationary operand. DoubleRow only
    supports PSUM quadrant 0, so the two 320-column halves of the l=2
    features accumulate into two separate PSUM tiles.
  * The MLP runs x0 (fp8) against bf16 weights; per-node scalars come from a
    1-wide W3 matmul packed into spare PE column bands.

Inputs stream as 5-group superchunks, each split into a 1-group head DMA
(fast pipeline ramp) plus a 4-group rest DMA (~2.6 MB, near-peak HBM
efficiency); outputs stage in SBUF and store once per superchunk. Per-group
PE issue order interleaves the l=2 passes around the MLP layers so the
in-order PE queue never waits on the activation engine.

Host epilogue: scatter-add of per-group segment partials, contraction with
w_l2, bincount of per-node scalars, and the tiny (G,9)@(9,9) basis change.
"""

import sys

if "/opt/trn_rl_repo" not in sys.path:
    sys.path.insert(0, "/opt/trn_rl_repo")

import numpy as np
import ml_dtypes

import concourse.bacc as bacc
import concourse.tile as tile
from concourse import mybir
from concourse import bass_utils

_S2 = 2.0 ** -0.5
_S3 = 3.0 ** -0.5
_S6 = 6.0 ** -0.5
_CG = np.array([
    [_S3, 0, 0, 0, _S3, 0, 0, 0, _S3],
    [0, 0, 0, 0, 0, _S2, 0, -_S2, 0],
    [0, 0, -_S2, 0, 0, 0, _S2, 0, 0],
    [0, _S2, 0, -_S2, 0, 0, 0, 0, 0],
    [0, 0, _S2, 0, 0, 0, _S2, 0, 0],
    [0, 0, 0, 0, 0, _S2, 0, _S2, 0],
    [-_S6, 0, 0, 0, 2 * _S6, 0, 0, 0, -_S6],
    [0, _S2, 0, _S2, 0, 0, 0, 0, 0],
    [-_S2, 0, 0, 0, 0, 0, 0, 0, _S2],
], dtype=np.float32)  # (9, 9)

N_CORES = 8
P = 128          # SBUF partitions
NG = 1024        # real nodes per group (one PSUM accumulation span)
DT = 2           # 256-supernode dtiles per group (supernode = 2 real nodes)
ML2 = 640        # l=2 values per node (5 m-rows x 128 channels)
SCG = 5          # groups per superchunk (one input DMA / output store)
GPB = 2560       # el2 bytes per partition per group (512 supernodes)

F32 = mybir.dt.float32
BF16 = mybir.dt.bfloat16
F8 = mybir.dt.float8e4
DR = mybir.MatmulPerfMode.DoubleRow
WIRE8 = ml_dtypes.float8_e4m3

_BUILD_CACHE = {}


def _build(n_groups, W, n_real):
    key = (n_groups, W, n_real)
    if key in _BUILD_CACHE:
        return _BUILD_CACHE[key]

    n_sc = (n_groups + SCG - 1) // SCG
    T2 = n_groups * DT * 2

    nc = bacc.Bacc("TRN2", target_bir_lowering=False, debug=False,
                   num_devices=N_CORES)

    # inputs (host pre-packed; see kernel() for layouts)
    el2w = nc.dram_tensor("el2w", (n_sc, P, SCG * GPB), F8,
                          kind="ExternalInput").ap()
    x0w = nc.dram_tensor("x0w", (n_sc, P, SCG * NG), F8,
                         kind="ExternalInput").ap()
    # consts packed into two tensors (one DMA each): bf16 weights and the
    # f32 block [b1 | b2 | iota(W) | lgid(T2)]
    wpk = nc.dram_tensor("wpk", (P, 2 * P), BF16, kind="ExternalInput").ap()
    fpk = nc.dram_tensor("fpk", (P, 2 + W + T2), F32,
                         kind="ExternalInput").ap()
    # outputs: l=2 segment partials (rows 0:W, halves at i*320), and the
    # fp8 hidden activations (host contracts with W3 for the scalar branch)
    S_out = nc.dram_tensor("S_out", (n_sc, 32, SCG * 640), BF16,
                           kind="ExternalOutput").ap()
    h2o = nc.dram_tensor("h2o", (n_groups, P, NG), F8,
                         kind="ExternalOutput").ap()

    silu = mybir.ActivationFunctionType.Silu
    eq = mybir.AluOpType.is_equal

    with tile.TileContext(nc) as tc:
        with (
            tc.tile_pool(name="const", bufs=1) as cpool,
            tc.tile_pool(name="el2h", bufs=3) as el2hp,
            tc.tile_pool(name="el2r", bufs=3) as el2rp,
            tc.tile_pool(name="x0p", bufs=3) as x0p,
            tc.tile_pool(name="hp", bufs=4) as hp,
            tc.tile_pool(name="stp", bufs=2) as stp,
            tc.tile_pool(name="ph1", bufs=1, space="PSUM") as ph1p,
            tc.tile_pool(name="ph2", bufs=1, space="PSUM") as ph2p,
            tc.tile_pool(name="pS", bufs=2, space="PSUM") as pSp,
        ):
            wpks = cpool.tile([P, 2 * P], BF16)
            fpks = cpool.tile([P, 2 + W + T2], F32)
            # every input rides the sync queue (the scalar-ring HWDGE is
            # strictly deprioritized behind sync); consts first, then all
            # superchunks [x0, el2 head, el2 rest] — the sync engine's
            # in-order buffer waits provide natural flow control
            nc.sync.dma_start(out=wpks[:], in_=wpk)
            nc.sync.dma_start(out=fpks[:], in_=fpk)
            w1s = wpks[:, 0:P]
            w2s = wpks[:, P:2 * P]
            b1s = fpks[:, 0:1]
            b2s = fpks[:, 1:2]
            iotas = fpks[:, 2:2 + W]
            lgids = fpks[:, 2 + W:]

            # all indicator pair-matrices up front (one DVE instruction):
            # Aall[p, (g,d,i), w] = (iota[w] == lgid[p, (g,d,i)])
            Aall = cpool.tile([P, T2 * W], F8)
            nc.vector.tensor_tensor(
                out=Aall[:].rearrange("p (t w) -> p t w", t=T2, w=W),
                in0=iotas.unsqueeze(1).to_broadcast([P, T2, W]),
                in1=lgids.unsqueeze(2).to_broadcast([P, T2, W]),
                op=eq)

            x0cs, el2hs, el2rs = [], [], []
            for sc in range(n_sc):
                sc_g0 = sc * SCG
                sc_ng = min(SCG, n_groups - sc_g0)
                last_g = sc_g0 + sc_ng - 1
                lg_real = min(NG, n_real - last_g * NG)
                dt_last = (lg_real + 511) // 512   # 256-supernode dtiles
                sr_last = (lg_real + 511) // 512
                x0c = x0p.tile([P, SCG * NG], F8, tag="x0c")
                xext = (sc_ng - 1) * NG + sr_last * 512
                nc.sync.dma_start(out=x0c[:, :xext], in_=x0w[sc][:, :xext])
                el2h = el2hp.tile([P, GPB], F8, tag="el2h")
                hext = GPB if sc_ng > 1 else dt_last * 1280
                nc.sync.dma_start(out=el2h[:, :hext],
                                  in_=el2w[sc][:, :hext])
                el2r = el2rp.tile([P, (SCG - 1) * GPB], F8, tag="el2r")
                if sc_ng > 1:
                    ext = (sc_ng - 2) * GPB + dt_last * 1280
                    nc.sync.dma_start(out=el2r[:, :ext],
                                      in_=el2w[sc][:, GPB: GPB + ext])
                x0cs.append(x0c)
                el2hs.append(el2h)
                el2rs.append(el2r)

            # software-pipelined compute: iteration `it` issues
            #   L1(it) -> L2(it-1) -> l2(it) -> W3(it-1) -> copy(it-1)
            # so every matmul's cross-engine inputs are a full group old and
            # the in-order PE queue never stalls on the activation engine.
            # Matmuls sharing a stationary are paired; the second reuses the
            # PE-resident weights (ldweights=False).
            Ssts = {}
            h1map, h2map, pSmap = {}, {}, {}
            for it in range(n_groups + 1):
                if it < n_groups:
                    g = it
                    sc = g // SCG
                    gl = g % SCG
                    grp_real = min(NG, n_real - g * NG)
                    Sr = (grp_real + 511) // 512
                    if gl == 0:
                        Sst = stp.tile([32, SCG * 640], BF16, tag="Sst")
                        Ssts[sc] = Sst

                    # ---- MLP layer 1 of group g ----
                    h1p = ph1p.tile([P, 1024], F32, tag="h1p")
                    for s in range(Sr):
                        nsl = slice(gl * NG + s * 512, gl * NG + (s + 1) * 512)
                        mm = nc.tensor.matmul(h1p[:, s * 512:(s + 1) * 512],
                                              w1s, x0cs[sc][:, nsl],
                                              start=True, stop=True)
                        if s > 0:
                            mm.ldweights = False
                    h1s = hp.tile([P, 1024], BF16, tag="h1s")
                    nc.scalar.activation(h1s[:, :Sr * 512],
                                         h1p[:, :Sr * 512], silu, bias=b1s)
                    h1map[g] = (h1s, Sr)

                if it >= 1:
                    # ---- MLP layer 2 of group it-1; fp8 h2 to the host ----
                    gp = it - 1
                    h1s_p, Sr_p = h1map.pop(gp)
                    h2p = ph2p.tile([P, 1024], F32, tag="h2p")
                    for s in range(Sr_p):
                        nc.tensor.matmul(
                            h2p[:, s * 512:(s + 1) * 512], w2s,
                            h1s_p[:, s * 512:(s + 1) * 512],
                            start=True, stop=True)
                    h2s = hp.tile([P, 1024], F8, tag="h2s")
                    nc.scalar.activation(h2s[:, :Sr_p * 512],
                                         h2p[:, :Sr_p * 512], silu,
                                         bias=b2s)
                    nc.gpsimd.dma_start(out=h2o[gp][:, :Sr_p * 512],
                                        in_=h2s[:, :Sr_p * 512])

                if it < n_groups:
                    # ---- l=2 segment sum of group g (DoubleRow fp8) ----
                    # operates on supernodes (host pre-summed node pairs);
                    # one 2-bank PSUM tile per group:
                    #   [0:W, 0:320]    l=2 feature half 0 (bank A)
                    #   [0:W, 512:832]  l=2 feature half 1 (bank B)
                    #   [32:34, 0:512]  W3 scalars chunk 0 (dup rows)
                    #   [32:34, 512:1024] W3 scalars chunk 1
                    Dr = (grp_real + 511) // 512
                    pS = pSp.tile([P, 1024], F32, tag="pS")
                    if gl == 0:
                        esrc = el2hs[sc]
                        ebase = 0
                    else:
                        esrc = el2rs[sc]
                        ebase = (gl - 1) * GPB
                    for d in range(Dr):
                        t2i = (g * DT + d) * 2
                        Ad = Aall[:, t2i * W: (t2i + 2) * W] \
                            .rearrange("p (i w) -> p i w", i=2, w=W)
                        base = ebase + d * 1280
                        nc.tensor.matmul(
                            pS[0:W, 0:320], Ad,
                            esrc[:, base: base + 640]
                                .rearrange("p (i f) -> p i f", i=2, f=320),
                            start=(d == 0), stop=(d == Dr - 1),
                            perf_mode=DR, tile_position=(0, 0))
                        mm = nc.tensor.matmul(
                            pS[0:W, 512:832], Ad,
                            esrc[:, base + 640: base + 1280]
                                .rearrange("p (i f) -> p i f", i=2, f=320),
                            start=(d == 0), stop=(d == Dr - 1),
                            perf_mode=DR, tile_position=(0, 0))
                        mm.ldweights = False
                    pSmap[g] = pS

                if it >= 1:
                    # ---- staging copy of group it-1 (strided, skips pad) ----
                    gp = it - 1
                    pS_p = pSmap.pop(gp)
                    scp = gp // SCG
                    glp = gp % SCG
                    nc.vector.tensor_copy(
                        out=Ssts[scp][:, glp * 640: (glp + 1) * 640]
                            .rearrange("p (i f) -> p i f", i=2, f=320),
                        in_=pS_p[0:32, 0:1024]
                            .rearrange("p (i f) -> p i f", i=2, f=512)
                            [:, :, 0:320])
                    if gp == n_groups - 1 or glp == SCG - 1:
                        nc.gpsimd.dma_start(out=S_out[scp],
                                            in_=Ssts.pop(scp)[:])

    nc.compile()
    _BUILD_CACHE[key] = nc
    return nc


def _next_pow2(x):
    p = 8
    while p < x:
        p *= 2
    return p


def _host_reference(node_embedding, W1, b1, W2, b2, W3, b3, w_l2, batch,
                    natoms):
    """Pure-numpy fallback (only used for pathological graph layouts)."""
    G = natoms.shape[0]
    inv = 1.0 / natoms.astype(np.float32)
    x = node_embedding[:, 0, :]
    h = x @ W1.T + b1
    h = h / (1.0 + np.exp(-h))
    h = h @ W2.T + b2
    h = h / (1.0 + np.exp(-h))
    ns = (h @ W3.T + b3)[:, 0]
    ok = (batch >= 0) & (batch < G)
    bok = batch[ok]
    iso = np.bincount(bok, weights=ns[ok], minlength=G).astype(np.float32) \
        * inv
    nl2 = np.einsum("nmc,c->nm", node_embedding[:, 4:9, :], w_l2[0])
    aniso = np.stack(
        [np.bincount(bok, weights=nl2[ok, m], minlength=G)
         for m in range(5)], axis=1).astype(np.float32) * inv[:, None]
    dec = np.concatenate([iso[:, None], np.zeros((G, 3), np.float32), aniso],
                         axis=1)
    return (dec @ _CG).reshape(-1, 3, 3).astype(np.float32)


def _chain4_quant(vals, seg):
    """fp8 E4M3 quantization with 4-element error-feedback chains.

    Within each aligned run of 4 rows, the rounding residual of row k is
    added to row k+1 before its rounding whenever both rows belong to the
    same segment, so the segment sum sees ~1 rounding error per chain
    instead of 4."""
    n = vals.shape[0]
    v = vals.reshape(n // 4, 4, ML2)
    b4 = seg.reshape(n // 4, 4)
    out = np.empty((n // 4, 4, ML2), WIRE8)
    carry = np.zeros((n // 4, ML2), np.float32)
    for k in range(4):
        tgt = v[:, k] + carry
        q = tgt.astype(WIRE8)
        out[:, k] = q
        if k < 3:
            same = (b4[:, k] == b4[:, k + 1]).astype(np.float32)[:, None]
            carry = (tgt - q.astype(np.float32)) * same
    return out.reshape(n, ML2)


def kernel(node_embedding, W1, b1, W2, b2, W3, b3, w_l2, batch, natoms):
    node_embedding = np.asarray(node_embedding, dtype=np.float32)
    W1 = np.asarray(W1, dtype=np.float32)
    b1 = np.asarray(b1, dtype=np.float32)
    W2 = np.asarray(W2, dtype=np.float32)
    b2 = np.asarray(b2, dtype=np.float32)
    W3 = np.asarray(W3, dtype=np.float32)
    b3 = np.asarray(b3, dtype=np.float32)
    w_l2 = np.asarray(w_l2, dtype=np.float32)
    batch = np.asarray(batch).astype(np.int64)
    natoms_in = np.asarray(natoms)

    N = node_embedding.shape[0]
    G = natoms_in.shape[0]
    n_sh = (N + N_CORES - 1) // N_CORES
    n_sh = ((n_sh + 3) // 4) * 4       # chain alignment
    n_groups = (n_sh + NG - 1) // NG
    n_pad = n_groups * NG
    n_sc = (n_groups + SCG - 1) // SCG

    if N % 8 != 0:
        return _host_reference(node_embedding, W1, b1, W2, b2, W3, b3,
                               w_l2, batch, natoms_in)

    # per-core shard ranges and per-group base graph ids
    shards = []
    W_need = 8
    for c in range(N_CORES):
        n0 = min(c * n_sh, N)
        n1 = min(n0 + n_sh, N)
        b = batch[n0:n1]
        nreal = n1 - n0
        gbase = np.zeros(n_groups, np.int64)
        for grp in range(n_groups):
            lo = grp * NG
            hi = min(lo + NG, nreal)
            if lo < nreal:
                gbase[grp] = b[lo]
                span = int(b[hi - 1] - b[lo] + 1)
                W_need = max(W_need, span)
        shards.append((n0, n1, b, gbase))
    W = _next_pow2(W_need)
    if (W > 32 or not np.all(batch[:-1] <= batch[1:])
            or batch.min(initial=0) < 0 or batch.max(initial=0) >= G):
        return _host_reference(node_embedding, W1, b1, W2, b2, W3, b3,
                               w_l2, batch, natoms_in)

    nc = _build(n_groups, W, n_sh)

    WIRE16 = ml_dtypes.bfloat16
    wpk = np.ascontiguousarray(
        np.concatenate([W1.T, W2.T], axis=1)).astype(WIRE16)
    iota_c = np.tile(np.arange(W, dtype=np.float32), (P, 1))
    fpk_base = np.concatenate(
        [b1[:, None], b2[:, None], iota_c], axis=1).astype(np.float32)

    # l=2 branch runs on supernodes: adjacent node pairs pre-summed on the
    # host when both land in the same graph; when a pair straddles a graph
    # boundary the odd node is exiled to an exact host-side correction.
    el2n = node_embedding[:, 4:9, :].reshape(N, ML2)
    bs = batch[0::2]
    same_pair = (bs == batch[1::2])
    sn = el2n[0::2] + el2n[1::2] * same_pair[:, None].astype(np.float32)
    el2q = _chain4_quant(sn, bs)
    x0q = node_embedding[:, 0, :].astype(WIRE8)

    in_maps = []
    for c in range(N_CORES):
        n0, n1, b, gbase = shards[c]
        nreal = n1 - n0
        n_pad_sc = n_sc * SCG * NG
        # x0 wire: [sc, c(128), node] channel-major
        x0T = np.zeros((P, n_pad_sc), WIRE8)
        x0T[:, :nreal] = x0q[n0:n1].T
        x0w = np.ascontiguousarray(
            x0T.reshape(P, n_sc, SCG * NG).transpose(1, 0, 2))
        # el2 wire over supernodes: supernode j = g*512 + d*256 + 2p + i
        # lives at [sc, p, (gl, d, h, i, f320)]
        sreal = nreal // 2
        el2 = np.zeros((n_pad_sc // 2, ML2), WIRE8)
        el2[:sreal] = el2q[n0 // 2: n1 // 2]
        el2 = el2.reshape(n_sc, SCG, DT, P, 2, 2, 320)
        el2 = np.ascontiguousarray(el2.transpose(0, 3, 1, 2, 5, 4, 6)
                                   .reshape(n_sc, P, SCG * GPB))
        # local graph ids per supernode (g, d, p, i)
        lg = np.full(n_pad // 2, -1.0, np.float32)
        lg[:sreal] = (bs[n0 // 2: n1 // 2]
                      - np.repeat(gbase, NG // 2)[:sreal]).astype(np.float32)
        lg_t = np.ascontiguousarray(
            lg.reshape(n_groups, DT, P, 2).transpose(2, 0, 1, 3)
              .reshape(P, n_groups * DT * 2))
        fpk = np.ascontiguousarray(
            np.concatenate([fpk_base, lg_t], axis=1))
        in_maps.append({
            "el2w": el2, "x0w": x0w, "wpk": wpk, "fpk": fpk,
        })

    res = bass_utils.run_bass_kernel_spmd(nc, in_maps,
                                          core_ids=list(range(N_CORES)))

    # ---- host epilogue ----
    inv = (1.0 / natoms_in.astype(np.float32)).astype(np.float32)
    node_scalar = np.empty(N, np.float32)
    Sfull = np.zeros((G + 32, ML2), np.float32)
    for c in range(N_CORES):
        n0, n1, _, gbase = shards[c]
        nreal = n1 - n0
        Sc = np.asarray(res.results[c]["S_out"]).astype(np.float32)
        # per-node scalars: host contraction of the fp8 hidden activations
        h2f = np.asarray(res.results[c]["h2o"]).astype(np.float32)
        sc_nodes = np.tensordot(W3[0], h2f, axes=([0], [1])).reshape(-1)
        node_scalar[n0:n1] = sc_nodes[:nreal]
        for grp in range(n_groups):
            if grp * NG < nreal:
                gb = int(gbase[grp])
                blk = Sc[grp // SCG][:, (grp % SCG) * 640:
                                     (grp % SCG + 1) * 640]
                Sfull[gb:gb + W, 0:320] += blk[0:W, 0:320]
                Sfull[gb:gb + W, 320:640] += blk[0:W, 320:640]
    iso = np.bincount(batch, weights=node_scalar + b3[0], minlength=G)
    iso = iso.astype(np.float32) * inv
    aniso = (Sfull[:G].reshape(G, 5, P) @ w_l2[0]).astype(np.float32)
    # exact host-side correction for nodes exiled from straddling pairs
    ex = np.nonzero(~same_pair)[0]
    if ex.size:
        exn = 2 * ex + 1
        nl2_ex = np.einsum("nmc,c->nm", node_embedding[exn, 4:9, :], w_l2[0])
        bex = batch[exn]
        for m in range(5):
            aniso[:, m] += np.bincount(
                bex, weights=nl2_ex[:, m], minlength=G).astype(np.float32)
    aniso *= inv[:, None]
    dec = np.concatenate([iso[:, None], np.zeros((G, 3), np.float32), aniso],
                         axis=1)
    return (dec @ _CG).reshape(-1, 3, 3).astype(np.float32)


# revision 47
# speedup vs baseline: 1.8910x; 1.1220x over previous
"""CartBasisStressHead kernel for Trainium2 (8 NeuronCores, SPMD data-parallel).

Strategy
--------
Only 6 of the 9 m-rows of node_embedding are used: row 0 feeds a SiLU MLP
(per-node scalar), rows 4:9 feed a per-channel contraction (l=2 branch).
Nodes are sharded contiguously across 8 cores; segment sums are linear, so
the host adds per-shard partials.

The kernel is HBM-bound, so everything streamed is wired in fp8 (E4M3):
  * l=2 data uses a chain-of-4 compensated quantization (each node's rounding
    residual is folded into the next node of the same graph before rounding),
    cutting the segment-sum quantization error ~2x below plain fp8 rounding.
  * The segment sum itself runs on the PE in DoubleRow fp8 perf mode:
    256 nodes per pass (128 partitions x 2 pair lanes), with a 0/1 indicator
    matrix A[node, local_graph] as the stationary operand. DoubleRow only
    supports PSUM quadrant 0, so the two 320-column halves of the l=2
    features accumulate into two separate PSUM tiles.
  * The MLP runs x0 (fp8) against bf16 weights; per-node scalars come from a
    1-wide W3 matmul packed into spare PE column bands.

Inputs stream as 5-group superchunks, each split into a 1-group head DMA
(fast pipeline ramp) plus a 4-group rest DMA (~2.6 MB, near-peak HBM
efficiency); outputs stage in SBUF and store once per superchunk. Per-group
PE issue order interleaves the l=2 passes around the MLP layers so the
in-order PE queue never waits on the activation engine.

Host epilogue: scatter-add of per-group segment partials, contraction with
w_l2, bincount of per-node scalars, and the tiny (G,9)@(9,9) basis change.
"""

import sys

if "/opt/trn_rl_repo" not in sys.path:
    sys.path.insert(0, "/opt/trn_rl_repo")

import numpy as np
import ml_dtypes

import concourse.bacc as bacc
import concourse.tile as tile
from concourse import mybir
from concourse import bass_utils

_S2 = 2.0 ** -0.5
_S3 = 3.0 ** -0.5
_S6 = 6.0 ** -0.5
_CG = np.array([
    [_S3, 0, 0, 0, _S3, 0, 0, 0, _S3],
    [0, 0, 0, 0, 0, _S2, 0, -_S2, 0],
    [0, 0, -_S2, 0, 0, 0, _S2, 0, 0],
    [0, _S2, 0, -_S2, 0, 0, 0, 0, 0],
    [0, 0, _S2, 0, 0, 0, _S2, 0, 0],
    [0, 0, 0, 0, 0, _S2, 0, _S2, 0],
    [-_S6, 0, 0, 0, 2 * _S6, 0, 0, 0, -_S6],
    [0, _S2, 0, _S2, 0, 0, 0, 0, 0],
    [-_S2, 0, 0, 0, 0, 0, 0, 0, _S2],
], dtype=np.float32)  # (9, 9)

N_CORES = 8
P = 128          # SBUF partitions
NG = 1024        # real nodes per group (one PSUM accumulation span)
DT = 2           # 256-supernode dtiles per group (supernode = 2 real nodes)
ML2 = 640        # l=2 values per node (5 m-rows x 128 channels)
SCG = 5          # groups per superchunk (one input DMA / output store)
GPB = 2560       # el2 bytes per partition per group (512 supernodes)

F32 = mybir.dt.float32
BF16 = mybir.dt.bfloat16
F8 = mybir.dt.float8e4
DR = mybir.MatmulPerfMode.DoubleRow
WIRE8 = ml_dtypes.float8_e4m3

_BUILD_CACHE = {}


def _build(n_groups, W, n_real):
    key = (n_groups, W, n_real)
    if key in _BUILD_CACHE:
        return _BUILD_CACHE[key]

    n_sc = (n_groups + SCG - 1) // SCG
    T2 = n_groups * DT * 2

    nc = bacc.Bacc("TRN2", target_bir_lowering=False, debug=False,
                   num_devices=N_CORES)

    # inputs (host pre-packed; see kernel() for layouts)
    el2w = nc.dram_tensor("el2w", (n_sc, P, SCG * GPB), F8,
                          kind="ExternalInput").ap()
    x0w = nc.dram_tensor("x0w", (n_sc, P, SCG * NG), F8,
                         kind="ExternalInput").ap()
    # consts packed into two tensors (one DMA each): bf16 weights and the
    # f32 block [b1 | b2 | iota(W) | lgid(T2)]
    wpk = nc.dram_tensor("wpk", (P, 2 * P), BF16, kind="ExternalInput").ap()
    fpk = nc.dram_tensor("fpk", (P, 2 + W + T2), F32,
                         kind="ExternalInput").ap()
    # outputs: l=2 segment partials (rows 0:W, halves at i*320), and the
    # fp8 hidden activations (host contracts with W3 for the scalar branch)
    S_out = nc.dram_tensor("S_out", (n_sc, 32, SCG * 640), BF16,
                           kind="ExternalOutput").ap()
    h2o = nc.dram_tensor("h2o", (n_groups, P, NG), F8,
                         kind="ExternalOutput").ap()

    silu = mybir.ActivationFunctionType.Silu
    eq = mybir.AluOpType.is_equal

    with tile.TileContext(nc) as tc:
        with (
            tc.tile_pool(name="const", bufs=1) as cpool,
            tc.tile_pool(name="el2p", bufs=8) as el2p,
            tc.tile_pool(name="x0p", bufs=3) as x0p,
            tc.tile_pool(name="hp", bufs=4) as hp,
            tc.tile_pool(name="stp", bufs=2) as stp,
            tc.tile_pool(name="ph1", bufs=1, space="PSUM") as ph1p,
            tc.tile_pool(name="ph2", bufs=1, space="PSUM") as ph2p,
            tc.tile_pool(name="pS", bufs=2, space="PSUM") as pSp,
        ):
            wpks = cpool.tile([P, 2 * P], BF16)
            fpks = cpool.tile([P, 2 + W + T2], F32)
            # every input rides the sync queue (the scalar-ring HWDGE is
            # strictly deprioritized behind sync); consts first, then all
            # superchunks [x0, el2 head, el2 rest] — the sync engine's
            # in-order buffer waits provide natural flow control
            nc.sync.dma_start(out=wpks[:], in_=wpk)
            nc.sync.dma_start(out=fpks[:], in_=fpk)
            w1s = wpks[:, 0:P]
            w2s = wpks[:, P:2 * P]
            b1s = fpks[:, 0:1]
            b2s = fpks[:, 1:2]
            iotas = fpks[:, 2:2 + W]
            lgids = fpks[:, 2 + W:]

            # all indicator pair-matrices up front (one DVE instruction):
            # Aall[p, (g,d,i), w] = (iota[w] == lgid[p, (g,d,i)])
            Aall = cpool.tile([P, T2 * W], F8)
            nc.vector.tensor_tensor(
                out=Aall[:].rearrange("p (t w) -> p t w", t=T2, w=W),
                in0=iotas.unsqueeze(1).to_broadcast([P, T2, W]),
                in1=lgids.unsqueeze(2).to_broadcast([P, T2, W]),
                op=eq)

            # need-ordered input stream: x0 slab per superchunk interleaved
            # with one el2 DMA per group (0.65 MB each); x0 of superchunk
            # k+1 is issued after the first el2 group of superchunk k so it
            # always lands well before its first MLP matmul
            x0cs = [x0p.tile([P, SCG * NG], F8, tag="x0c",
                             name=f"x0c{k}") for k in range(n_sc)]
            el2gs = [el2p.tile([P, GPB], F8, tag="el2g",
                               name=f"el2g{k}") for k in range(n_groups)]

            def xext_of(sc):
                sc_ng = min(SCG, n_groups - sc * SCG)
                lg_real = min(NG, n_real - (sc * SCG + sc_ng - 1) * NG)
                return (sc_ng - 1) * NG + ((lg_real + 511) // 512) * 512

            def dma_el2(g):
                grp_real = min(NG, n_real - g * NG)
                gext = ((grp_real + 511) // 512) * 1280
                nc.sync.dma_start(
                    out=el2gs[g][:, :gext],
                    in_=el2w[g // SCG][:, (g % SCG) * GPB:
                                       (g % SCG) * GPB + gext])

            nc.sync.dma_start(out=x0cs[0][:, :NG], in_=x0w[0][:, :NG])
            dma_el2(0)
            nc.sync.dma_start(out=x0cs[0][:, NG:xext_of(0)],
                              in_=x0w[0][:, NG:xext_of(0)])
            for g in range(1, n_groups):
                dma_el2(g)
                if g % SCG == 1 and g // SCG + 1 < n_sc:
                    nxt = g // SCG + 1
                    nc.sync.dma_start(out=x0cs[nxt][:, :xext_of(nxt)],
                                      in_=x0w[nxt][:, :xext_of(nxt)])

            # software-pipelined compute: iteration `it` issues
            #   L1(it) -> L2(it-1) -> l2(it) -> W3(it-1) -> copy(it-1)
            # so every matmul's cross-engine inputs are a full group old and
            # the in-order PE queue never stalls on the activation engine.
            # Matmuls sharing a stationary are paired; the second reuses the
            # PE-resident weights (ldweights=False).
            Ssts = {}
            h1map, h2map, pSmap = {}, {}, {}
            for it in range(n_groups + 1):
                if it < n_groups:
                    g = it
                    sc = g // SCG
                    gl = g % SCG
                    grp_real = min(NG, n_real - g * NG)
                    Sr = (grp_real + 511) // 512
                    if gl == 0:
                        Sst = stp.tile([32, SCG * 640], BF16, tag="Sst")
                        Ssts[sc] = Sst

                    # ---- MLP layer 1 of group g ----
                    h1p = ph1p.tile([P, 1024], F32, tag="h1p")
                    for s in range(Sr):
                        nsl = slice(gl * NG + s * 512, gl * NG + (s + 1) * 512)
                        mm = nc.tensor.matmul(h1p[:, s * 512:(s + 1) * 512],
                                              w1s, x0cs[sc][:, nsl],
                                              start=True, stop=True)
                        if s > 0:
                            mm.ldweights = False
                    h1s = hp.tile([P, 1024], BF16, tag="h1s")
                    nc.scalar.activation(h1s[:, :Sr * 512],
                                         h1p[:, :Sr * 512], silu, bias=b1s)
                    h1map[g] = (h1s, Sr)

                if it >= 1:
                    # ---- MLP layer 2 of group it-1; fp8 h2 to the host ----
                    gp = it - 1
                    h1s_p, Sr_p = h1map.pop(gp)
                    h2p = ph2p.tile([P, 1024], F32, tag="h2p")
                    for s in range(Sr_p):
                        nc.tensor.matmul(
                            h2p[:, s * 512:(s + 1) * 512], w2s,
                            h1s_p[:, s * 512:(s + 1) * 512],
                            start=True, stop=True)
                    h2s = hp.tile([P, 1024], F8, tag="h2s")
                    nc.scalar.activation(h2s[:, :Sr_p * 512],
                                         h2p[:, :Sr_p * 512], silu,
                                         bias=b2s)
                    nc.gpsimd.dma_start(out=h2o[gp][:, :Sr_p * 512],
                                        in_=h2s[:, :Sr_p * 512])

                if it < n_groups:
                    # ---- l=2 segment sum of group g (DoubleRow fp8) ----
                    # operates on supernodes (host pre-summed node pairs);
                    # one 2-bank PSUM tile per group:
                    #   [0:W, 0:320]    l=2 feature half 0 (bank A)
                    #   [0:W, 512:832]  l=2 feature half 1 (bank B)
                    #   [32:34, 0:512]  W3 scalars chunk 0 (dup rows)
                    #   [32:34, 512:1024] W3 scalars chunk 1
                    Dr = (grp_real + 511) // 512
                    pS = pSp.tile([P, 1024], F32, tag="pS")
                    esrc = el2gs[g]
                    ebase = 0
                    for d in range(Dr):
                        t2i = (g * DT + d) * 2
                        Ad = Aall[:, t2i * W: (t2i + 2) * W] \
                            .rearrange("p (i w) -> p i w", i=2, w=W)
                        base = ebase + d * 1280
                        nc.tensor.matmul(
                            pS[0:W, 0:320], Ad,
                            esrc[:, base: base + 640]
                                .rearrange("p (i f) -> p i f", i=2, f=320),
                            start=(d == 0), stop=(d == Dr - 1),
                            perf_mode=DR, tile_position=(0, 0))
                        mm = nc.tensor.matmul(
                            pS[0:W, 512:832], Ad,
                            esrc[:, base + 640: base + 1280]
                                .rearrange("p (i f) -> p i f", i=2, f=320),
                            start=(d == 0), stop=(d == Dr - 1),
                            perf_mode=DR, tile_position=(0, 0))
                        mm.ldweights = False
                    pSmap[g] = pS

                if it >= 1:
                    # ---- staging copy of group it-1 (strided, skips pad) ----
                    gp = it - 1
                    pS_p = pSmap.pop(gp)
                    scp = gp // SCG
                    glp = gp % SCG
                    nc.vector.tensor_copy(
                        out=Ssts[scp][:, glp * 640: (glp + 1) * 640]
                            .rearrange("p (i f) -> p i f", i=2, f=320),
                        in_=pS_p[0:32, 0:1024]
                            .rearrange("p (i f) -> p i f", i=2, f=512)
                            [:, :, 0:320])
                    if gp == n_groups - 1 or glp == SCG - 1:
                        nc.gpsimd.dma_start(out=S_out[scp],
                                            in_=Ssts.pop(scp)[:])

    nc.compile()
    _BUILD_CACHE[key] = nc
    return nc


def _next_pow2(x):
    p = 8
    while p < x:
        p *= 2
    return p


def _host_reference(node_embedding, W1, b1, W2, b2, W3, b3, w_l2, batch,
                    natoms):
    """Pure-numpy fallback (only used for pathological graph layouts)."""
    G = natoms.shape[0]
    inv = 1.0 / natoms.astype(np.float32)
    x = node_embedding[:, 0, :]
    h = x @ W1.T + b1
    h = h / (1.0 + np.exp(-h))
    h = h @ W2.T + b2
    h = h / (1.0 + np.exp(-h))
    ns = (h @ W3.T + b3)[:, 0]
    ok = (batch >= 0) & (batch < G)
    bok = batch[ok]
    iso = np.bincount(bok, weights=ns[ok], minlength=G).astype(np.float32) \
        * inv
    nl2 = np.einsum("nmc,c->nm", node_embedding[:, 4:9, :], w_l2[0])
    aniso = np.stack(
        [np.bincount(bok, weights=nl2[ok, m], minlength=G)
         for m in range(5)], axis=1).astype(np.float32) * inv[:, None]
    dec = np.concatenate([iso[:, None], np.zeros((G, 3), np.float32), aniso],
                         axis=1)
    return (dec @ _CG).reshape(-1, 3, 3).astype(np.float32)


def _chain4_quant(vals, seg):
    """fp8 E4M3 quantization with 4-element error-feedback chains.

    Within each aligned run of 4 rows, the rounding residual of row k is
    added to row k+1 before its rounding whenever both rows belong to the
    same segment, so the segment sum sees ~1 rounding error per chain
    instead of 4."""
    n = vals.shape[0]
    v = vals.reshape(n // 4, 4, ML2)
    b4 = seg.reshape(n // 4, 4)
    out = np.empty((n // 4, 4, ML2), WIRE8)
    carry = np.zeros((n // 4, ML2), np.float32)
    for k in range(4):
        tgt = v[:, k] + carry
        q = tgt.astype(WIRE8)
        out[:, k] = q
        if k < 3:
            same = (b4[:, k] == b4[:, k + 1]).astype(np.float32)[:, None]
            carry = (tgt - q.astype(np.float32)) * same
    return out.reshape(n, ML2)


def kernel(node_embedding, W1, b1, W2, b2, W3, b3, w_l2, batch, natoms):
    node_embedding = np.asarray(node_embedding, dtype=np.float32)
    W1 = np.asarray(W1, dtype=np.float32)
    b1 = np.asarray(b1, dtype=np.float32)
    W2 = np.asarray(W2, dtype=np.float32)
    b2 = np.asarray(b2, dtype=np.float32)
    W3 = np.asarray(W3, dtype=np.float32)
    b3 = np.asarray(b3, dtype=np.float32)
    w_l2 = np.asarray(w_l2, dtype=np.float32)
    batch = np.asarray(batch).astype(np.int64)
    natoms_in = np.asarray(natoms)

    N = node_embedding.shape[0]
    G = natoms_in.shape[0]
    n_sh = (N + N_CORES - 1) // N_CORES
    n_sh = ((n_sh + 3) // 4) * 4       # chain alignment
    n_groups = (n_sh + NG - 1) // NG
    n_pad = n_groups * NG
    n_sc = (n_groups + SCG - 1) // SCG

    if N % 8 != 0:
        return _host_reference(node_embedding, W1, b1, W2, b2, W3, b3,
                               w_l2, batch, natoms_in)

    # per-core shard ranges and per-group base graph ids
    shards = []
    W_need = 8
    for c in range(N_CORES):
        n0 = min(c * n_sh, N)
        n1 = min(n0 + n_sh, N)
        b = batch[n0:n1]
        nreal = n1 - n0
        gbase = np.zeros(n_groups, np.int64)
        for grp in range(n_groups):
            lo = grp * NG
            hi = min(lo + NG, nreal)
            if lo < nreal:
                gbase[grp] = b[lo]
                span = int(b[hi - 1] - b[lo] + 1)
                W_need = max(W_need, span)
        shards.append((n0, n1, b, gbase))
    W = _next_pow2(W_need)
    if (W > 32 or not np.all(batch[:-1] <= batch[1:])
            or batch.min(initial=0) < 0 or batch.max(initial=0) >= G):
        return _host_reference(node_embedding, W1, b1, W2, b2, W3, b3,
                               w_l2, batch, natoms_in)

    nc = _build(n_groups, W, n_sh)

    WIRE16 = ml_dtypes.bfloat16
    wpk = np.ascontiguousarray(
        np.concatenate([W1.T, W2.T], axis=1)).astype(WIRE16)
    iota_c = np.tile(np.arange(W, dtype=np.float32), (P, 1))
    fpk_base = np.concatenate(
        [b1[:, None], b2[:, None], iota_c], axis=1).astype(np.float32)

    # l=2 branch runs on supernodes: adjacent node pairs pre-summed on the
    # host when both land in the same graph; when a pair straddles a graph
    # boundary the odd node is exiled to an exact host-side correction.
    el2n = node_embedding[:, 4:9, :].reshape(N, ML2)
    bs = batch[0::2]
    same_pair = (bs == batch[1::2])
    sn = el2n[0::2] + el2n[1::2] * same_pair[:, None].astype(np.float32)
    el2q = _chain4_quant(sn, bs)
    x0q = node_embedding[:, 0, :].astype(WIRE8)

    in_maps = []
    for c in range(N_CORES):
        n0, n1, b, gbase = shards[c]
        nreal = n1 - n0
        n_pad_sc = n_sc * SCG * NG
        # x0 wire: [sc, c(128), node] channel-major
        x0T = np.zeros((P, n_pad_sc), WIRE8)
        x0T[:, :nreal] = x0q[n0:n1].T
        x0w = np.ascontiguousarray(
            x0T.reshape(P, n_sc, SCG * NG).transpose(1, 0, 2))
        # el2 wire over supernodes: supernode j = g*512 + d*256 + 2p + i
        # lives at [sc, p, (gl, d, h, i, f320)]
        sreal = nreal // 2
        el2 = np.zeros((n_pad_sc // 2, ML2), WIRE8)
        el2[:sreal] = el2q[n0 // 2: n1 // 2]
        el2 = el2.reshape(n_sc, SCG, DT, P, 2, 2, 320)
        el2 = np.ascontiguousarray(el2.transpose(0, 3, 1, 2, 5, 4, 6)
                                   .reshape(n_sc, P, SCG * GPB))
        # local graph ids per supernode (g, d, p, i)
        lg = np.full(n_pad // 2, -1.0, np.float32)
        lg[:sreal] = (bs[n0 // 2: n1 // 2]
                      - np.repeat(gbase, NG // 2)[:sreal]).astype(np.float32)
        lg_t = np.ascontiguousarray(
            lg.reshape(n_groups, DT, P, 2).transpose(2, 0, 1, 3)
              .reshape(P, n_groups * DT * 2))
        fpk = np.ascontiguousarray(
            np.concatenate([fpk_base, lg_t], axis=1))
        in_maps.append({
            "el2w": el2, "x0w": x0w, "wpk": wpk, "fpk": fpk,
        })

    res = bass_utils.run_bass_kernel_spmd(nc, in_maps,
                                          core_ids=list(range(N_CORES)))

    # ---- host epilogue ----
    inv = (1.0 / natoms_in.astype(np.float32)).astype(np.float32)
    node_scalar = np.empty(N, np.float32)
    Sfull = np.zeros((G + 32, ML2), np.float32)
    for c in range(N_CORES):
        n0, n1, _, gbase = shards[c]
        nreal = n1 - n0
        Sc = np.asarray(res.results[c]["S_out"]).astype(np.float32)
        # per-node scalars: host contraction of the fp8 hidden activations
        h2f = np.asarray(res.results[c]["h2o"]).astype(np.float32)
        sc_nodes = np.tensordot(W3[0], h2f, axes=([0], [1])).reshape(-1)
        node_scalar[n0:n1] = sc_nodes[:nreal]
        for grp in range(n_groups):
            if grp * NG < nreal:
                gb = int(gbase[grp])
                blk = Sc[grp // SCG][:, (grp % SCG) * 640:
                                     (grp % SCG + 1) * 640]
                Sfull[gb:gb + W, 0:320] += blk[0:W, 0:320]
                Sfull[gb:gb + W, 320:640] += blk[0:W, 320:640]
    iso = np.bincount(batch, weights=node_scalar + b3[0], minlength=G)
    iso = iso.astype(np.float32) * inv
    aniso = (Sfull[:G].reshape(G, 5, P) @ w_l2[0]).astype(np.float32)
    # exact host-side correction for nodes exiled from straddling pairs
    ex = np.nonzero(~same_pair)[0]
    if ex.size:
        exn = 2 * ex + 1
        nl2_ex = np.einsum("nmc,c->nm", node_embedding[exn, 4:9, :], w_l2[0])
        bex = batch[exn]
        for m in range(5):
            aniso[:, m] += np.bincount(
                bex, weights=nl2_ex[:, m], minlength=G).astype(np.float32)
    aniso *= inv[:, None]
    dec = np.concatenate([iso[:, None], np.zeros((G, 3), np.float32), aniso],
                         axis=1)
    return (dec @ _CG).reshape(-1, 3, 3).astype(np.float32)
